# revision 18
# baseline (speedup 1.0000x reference)
"""Trainium2 Bass kernel for the 9-layer dense MLP (dropout-mask training forward).

Strategy (pure data parallel, 8 cores, 8192 batch rows each):
  - Activations kept transposed on-chip: features on partitions, batch cols on free dim.
    Each layer computes zT = W^T @ hT via nc.tensor.matmul(out, lhsT=W, rhs=hT).
  - fp16 weights/activations/masks (fp32 PSUM accumulation), fp32 biases + output.
  - Dropout masks binarized on host ({0,1} fp16); the 1/keep scale is folded into the
    next layer's weights.
  - Host pack layout [NBLK, 128, NPACK, BLK] so each per-block DMA is contiguous per
    partition (2 DMAs per block: x+m1, then the remaining masks).
  - PSUM: one shared pool of 4x [128,1024] fp32 tiles (8 banks). Matmuls write 512-col
    halves; drains are single FD=1024 instructions (fused bias+relu) split ~3:1
    ACT:DVE; mask multiplies are FD=2048 DVE tensor_tensor with a measured dose
    offloaded to GpSimd.
  - Small layers 6/7/8 partition-packed (offsets 0/64/96 via matmul tile_position);
    each ladder step drains immediately to SBUF so its PSUM tile recycles fast.
    Block b's ladder is software-pipelined into block b+1's big-layer bursts.
  - A short burst of dummy matmuls at t~1us keeps the PE HAM activity monitor busy so
    the array is at full clock (K=8/8) when real work arrives.
"""

import sys

sys.path.insert(0, "/opt/trn_rl_repo")

import numpy as np

DIMS = [256, 128, 256, 512, 256, 128, 64, 32, 16, 10]
NCORES = 8
BATCH = 65536
SHARD = BATCH // NCORES  # 8192
MSUB = 512               # matmul N (PSUM bank limit for fp32)
DSUB = 1024              # drain granularity (2 banks)
BLK = 2048               # block columns
NBLK = SHARD // BLK      # 4
NSUB = BLK // MSUB       # 4
NDR = BLK // DSUB        # 2

# pack chunk layout (each chunk = 128 partitions x BLK cols, fp16), per block:
#   0,1: xT   2: m1   3,4: m2   5-8: m3   9,10: m4   11: m5
# m678 ships separately as NBLK+2 slot-staggered planes (rows 0:64 = m6(slot),
# 64:96 = m7(slot-1), 96:112 = m8(slot-2)) to match the slot-fused ladder.
NPACK = 12
NSLOT = NBLK + 2

_PROG = {}


def _raise_sbuf_cap():
    # tile_utils.max_sbuf_usage is a stale 192KB constant; cayman has 208KB usable.
    import concourse.tile_utils as tu

    if getattr(tu, "max_sbuf_usage", 0) < 206 * 1024:
        tu.max_sbuf_usage = 206 * 1024


def _dedup_ldweights(nc):
    """Remove back-to-back redundant LDWEIGHTS (same stationary operand) so
    consecutive same-weight matmuls pipeline on the PE. Only drops LDW
    instructions that carry no semaphore waits/updates."""
    removed = 0
    for fn in nc.m.functions:
        for blk in fn.blocks:
            il = blk.instructions
            keep, last_sig = [], None
            for inst in il:
                nm = type(inst).__name__
                if nm == "InstLdweights":
                    sig = (str(inst.ins[0]), str(inst.is_transpose), str(inst.perf_mode),
                           str(getattr(inst, "tile_position", None)))
                    si = inst.sync_info
                    clean = si is None or (not si.on_wait and not si.on_update)
                    if sig == last_sig and clean:
                        removed += 1
                        continue
                    last_sig = sig
                keep.append(inst)
            if removed and len(keep) != len(il):
                while il:
                    il.pop()
                il.extend(keep)
    return removed


def _build_program():
    import concourse.bass as bass
    import concourse.tile as tile
    from concourse import bacc, mybir

    _raise_sbuf_cap()

    f16 = mybir.dt.float16
    f32 = mybir.dt.float32
    RELU = mybir.ActivationFunctionType.Relu
    IDENT = mybir.ActivationFunctionType.Identity
    ADD = mybir.AluOpType.add
    MAX = mybir.AluOpType.max

    nc = bacc.Bacc("TRN2", target_bir_lowering=False, debug=False, num_devices=NCORES)

    pack_d = nc.dram_tensor("pack", [NBLK, 128, NPACK, BLK], f16, kind="ExternalInput").ap()
    m678_d = nc.dram_tensor("M678", [NSLOT, 128, 1, BLK], f16, kind="ExternalInput").ap()
    # weights in two host-laid-out fp16 blobs (W1 separate so the first
    # LDWEIGHTS isn't gated on the full blob), biases in one fp32 blob
    wb1_d = nc.dram_tensor("WB1", [128, 256], f16, kind="ExternalInput").ap()
    wb_d = nc.dram_tensor("WB", [128, 2688], f16, kind="ExternalInput").ap()
    bb_d = nc.dram_tensor("BB", [128, 12], f32, kind="ExternalInput").ap()
    out_d = nc.dram_tensor("outT", [10, SHARD], f32, kind="ExternalOutput").ap()

    with tile.TileContext(nc) as tc:
        with (
            tc.tile_pool(name="wpool", bufs=1) as wp,
            tc.tile_pool(name="mk", bufs=2) as mkp,
            tc.tile_pool(name="hr", bufs=1) as hrp,
            tc.tile_pool(name="osb", bufs=2) as outp,
            tc.tile_pool(name="ps", bufs=4, space="PSUM") as psp,
        ):
            wall = wp.tile([128, 2944], f16, tag="wall")
            ball = wp.tile([128, 12], f32, tag="ball")
            scratch = wp.tile([128, 512], f16, tag="scratch")
            # blob column offsets: w1@0(256) w2@256(256) w3@512(1024) w4@1536(1024)
            #   w5@2560(256) w6@2816(64) w789@2880(64: W7 r0-63 c0-31, W8 r64-95
            #   c32-47, W9 r96-111 c48-57)
            WOFF = {1: 0, 2: 256, 3: 512, 4: 1536, 5: 2560, 6: 2816, 789: 2880}
            w789 = wall[:, WOFF[789]:WOFF[789] + 64]
            b15 = ball[:, 0:10]
            b678 = ball[:, 10:11]
            b9 = ball[0:10, 11:12]

            def wslice(l, k, c, N):
                base = WOFF[l] + k * N
                return wall[:, base + c * 128: base + (c + 1) * 128]

            def drain_relu(eng, dst, zsrc, bias_ap):
                if eng == "act":
                    nc.scalar.activation(dst, zsrc, RELU, bias=bias_ap)
                else:
                    nc.vector.tensor_scalar(dst, zsrc, bias_ap, 0.0, ADD, MAX)

            def mask_mul(eng, dst, src, msrc):
                if eng == "gps":
                    nc.gpsimd.tensor_mul(dst, src, msrc)
                else:
                    nc.vector.tensor_mul(dst, src, msrc)

            # drain engine picker: ~70:30 act:dve (ACT is cheaper per element but
            # DVE has mask work too; this balances their queues)
            dr_i = [0]

            def pick_drain():
                i = dr_i[0]
                dr_i[0] += 1
                return "dve" if i % 10 in (2, 5, 8) else "act"

            state = {}
            packs = {}
            hrs = {}

            def issue_pack_dmas(b):
                # per-chunk tiles/DMAs: each mask tile's ring slot is released as
                # soon as its own layer consumes it, so block b+2's DMAs start
                # early instead of waiting for ALL of block b's masks (WAR).
                pk3 = mkp.tile([128, 3, BLK], f16, tag="pk3", name=f"pk3_{b}")
                m2t = mkp.tile([128, 2, BLK], f16, tag="m2", name=f"m2_{b}")
                m3t = mkp.tile([128, 4, BLK], f16, tag="m3", name=f"m3_{b}")
                m4t = mkp.tile([128, 2, BLK], f16, tag="m4", name=f"m4_{b}")
                m5t = mkp.tile([128, 1, BLK], f16, tag="m5", name=f"m5_{b}")
                if b == 0:
                    nc.sync.dma_start(wall[:, 0:256], wb1_d[:])
                    nc.sync.dma_start(ball[:], bb_d[:])
                    nc.sync.dma_start(pk3[:, :, 0:DSUB], pack_d[0, :, 0:3, 0:DSUB])
                    nc.sync.dma_start(pk3[:, :, DSUB:BLK], pack_d[0, :, 0:3, DSUB:BLK])
                    nc.sync.dma_start(wall[:, 256:], wb_d[:])
                else:
                    nc.sync.dma_start(pk3[:], pack_d[b, :, 0:3, :])
                nc.sync.dma_start(m2t[:], pack_d[b, :, 3:5, :])
                nc.sync.dma_start(m3t[:], pack_d[b, :, 5:9, :])
                nc.sync.dma_start(m4t[:], pack_d[b, :, 9:11, :])
                nc.sync.dma_start(m5t[:], pack_d[b, :, 11:12, :])
                packs[b] = (pk3, m2t, m3t, m4t, m5t)
                issue_m678_dma(b)

            m678s = {}

            def issue_m678_dma(i):
                m678s[i] = mkp.tile([128, 1, BLK], f16, tag="m678", name=f"m678s_{i}",
                                    bufs=3)
                nc.sync.dma_start(m678s[i][:], m678_d[i])

            # (Kc, layer, wN, Cc, bias_off, hrtag)
            LAYER_CFG = [
                (2, 1, 128, 1, 0, "hr1"),
                (1, 2, 256, 2, 1, "hr2"),
                (2, 3, 512, 4, 3, "hr3"),
                (4, 4, 256, 2, 7, "hr4"),
                (2, 5, 128, 1, 9, "hr5"),
            ]
            # mask engine schedule per layer index. GpSimd is ~4x slower than DVE
            # per element, so it only gets masks with slack before their consumer:
            # m1 (L1 runs a block ahead) and m5 (ladder consumes it next block).
            # L2/L3/L4 masks sit on the next layer's critical path -> DVE only.
            MASK_MODE = {
                0: ["dve2"],
                1: ["dve", "dve"],
                2: ["dve", "dve", "dve", "dve"],
                3: ["dve", "dve"],
                4: ["dve2"],
            }
            # per-layer drain engine assignment, in (u, c) order. ACT-heavy for
            # the mid-block bulk (L3/L4); the boundary-critical L2 keeps one on
            # DVE so its chain isn't stuck behind the ACT queue.
            DRAIN_ENG = {
                0: ["dve", "act"],
                1: ["act", "act", "dve", "act"],
                2: ["act", "act", "act", "dve", "act", "act", "act", "act"],
                3: ["act", "act", "act", "act"],
                4: ["dve", "act"],
            }

            def emit_layer(b, li):
                Kc, wl, wN, Cc, boff, hrtag = LAYER_CFG[li]
                hr = hrp.tile([128, Cc, BLK], f16, tag=hrtag, name=hrtag + f"_{b}",
                              bufs=2 if hrtag in ("hr5", "hr2", "hr1") else 1)
                pk3 = packs[b][0]
                hin = pk3 if li == 0 else hrs[(b, li - 1)]

                def msl(c, cols):
                    if li == 0:
                        return pk3[:, 2, cols]
                    return packs[b][li][:, c, cols]

                zs = {}
                for c in range(Cc):
                    for u in range(NDR):
                        zs[c, u] = psp.tile([128, DSUB], f32, tag="ps",
                                            name=f"z_{hrtag}_{b}_{c}_{u}")
                if b == 0 and li == 0:
                    # startup: u-outer so each 1024-col window flows MM -> drain
                    # -> mask as early as possible
                    for u in range(NDR):
                        for k in range(Kc):
                            for t in range(2):
                                nc.tensor.matmul(
                                    zs[0, u][:, bass.ts(t, MSUB)],
                                    wslice(wl, k, 0, wN),
                                    hin[:, k, u * DSUB + t * MSUB:
                                        u * DSUB + (t + 1) * MSUB],
                                    start=(k == 0), stop=(k == Kc - 1))
                        drain_relu("dve" if u == 0 else "act",
                                   hr[:, 0, bass.ts(u, DSUB)], zs[0, u][:],
                                   b15[:, 0:1])
                        mask_mul("dve", hr[:, 0, bass.ts(u, DSUB)],
                                 hr[:, 0, bass.ts(u, DSUB)],
                                 msl(0, bass.ts(u, DSUB)))
                    hrs[(b, li)] = hr
                    return
                # weight-major matmuls so consecutive MMs share one LDWEIGHTS
                for c in range(Cc):
                    for k in range(Kc):
                        wap = wslice(wl, k, c, wN)
                        for t in range(NSUB):
                            nc.tensor.matmul(
                                zs[c, t // 2][:, bass.ts(t % 2, MSUB)], wap,
                                hin[:, k, bass.ts(t, MSUB)],
                                start=(k == 0), stop=(k == Kc - 1))
                de = DRAIN_ENG[li]
                for u in range(NDR):
                    for c in range(Cc):
                        drain_relu(de[u * Cc + c], hr[:, c, bass.ts(u, DSUB)],
                                   zs[c, u][:], b15[:, boff + c:boff + c + 1])
                full = slice(0, BLK)
                for c in range(Cc):
                    mode = MASK_MODE[li][c]
                    if mode == "dve":
                        mask_mul("dve", hr[:, c, full], hr[:, c, full], msl(c, full))
                    elif mode == "dve2":
                        for uu in range(NDR):
                            hs = bass.ts(uu, DSUB)
                            mask_mul("dve", hr[:, c, hs], hr[:, c, hs], msl(c, hs))
                    else:
                        hs0, hs1 = bass.ts(0, DSUB), bass.ts(1, DSUB)
                        mask_mul("dve", hr[:, c, hs0], hr[:, c, hs0], msl(c, hs0))
                        mask_mul("gps", hr[:, c, hs1], hr[:, c, hs1], msl(c, hs1))
                hrs[(b, li)] = hr
                if li > 0:
                    del hrs[(b, li - 1)]

            # --- small-layer ladder, slot-fused across blocks --------------------
            # Slot i co-issues three INDEPENDENT small matmuls from staggered
            # blocks into disjoint partition ranges of ONE PSUM tile:
            #   rows 0:64   L6(block i)    cols 0:64   of the PE array
            #   rows 64:96  L7(block i-1)  cols 64:96  (tile_position (0,64))
            #   rows 96:112 L8(block i-2)  cols 96:128 (tile_position (64,96))
            # They run concurrently (disjoint subarrays), and ONE drain + ONE
            # mask serves all three (the host staggers the m678 mask planes the
            # same way). L9(block i-2) then reads rows 96:112 after the mask.
            slots = {}
            hm5s = {}

            def slot_wins(fine):
                return (range(NSUB), MSUB) if fine else (range(NDR), DSUB)

            def emit_slot_trio(i, fine=False):
                sl = slots.setdefault(i, {})
                sl["hr678"] = hrp.tile([128, 1, BLK], f16, tag="hr678",
                                       name=f"hr678_{i}", bufs=2)
                prev = slots.get(i - 1)
                wins, wsz = slot_wins(fine)
                nmm = wsz // MSUB
                zhs = {}
                for u in wins:
                    zhs[u] = psp.tile([128, wsz], f32, tag="ps", name=f"zh_{i}_{u}")
                sl["zhs"] = zhs
                sl["fine"] = fine
                parts = []
                if i <= NBLK - 1:
                    parts.append((0, 64, wall[:, WOFF[6]:WOFF[6] + 64], None,
                                  lambda cs: hm5s[i][:, 0, cs]))
                if 1 <= i <= NBLK:
                    parts.append((64, 96, w789[0:64, 0:32], (0, 64),
                                  lambda cs: prev["hr678"][0:64, 0, cs]))
                if 2 <= i <= NBLK + 1:
                    parts.append((96, 112, w789[64:96, 32:48], (64, 96),
                                  lambda cs: prev["hr678"][64:96, 0, cs]))
                sl["p_lo"] = parts[0][0]
                # weight-major: one LDW per weight per slot (consecutive dedup),
                # streams of different col-groups overlap on the PE
                for p0, p1, wap, tpos, rhs_of in parts:
                    for u in wins:
                        for t in range(nmm):
                            cs = slice(u * wsz + t * MSUB, u * wsz + (t + 1) * MSUB)
                            out_sl = zhs[u][p0:p1, bass.ts(t, MSUB)]
                            if tpos is None:
                                nc.tensor.matmul(out_sl, wap, rhs_of(cs),
                                                 start=True, stop=True)
                            else:
                                nc.tensor.matmul(out_sl, wap, rhs_of(cs),
                                                 start=True, stop=True,
                                                 tile_position=tpos)
                if i in hm5s:
                    del hm5s[i]

            def emit_slot_drainmask(i):
                sl = slots[i]
                p_lo = sl["p_lo"]
                hr678 = sl["hr678"]
                m678 = m678s[i]
                wins, wsz = slot_wins(sl["fine"])
                for u in wins:
                    dst = hr678[p_lo:112, 0, u * wsz:(u + 1) * wsz]
                    drain_relu("dve" if u % 2 == 0 else "act", dst,
                               sl["zhs"][u][p_lo:112, :], b678[p_lo:112, 0:1])
                    # hs1 half -> gps only at coarse granularity (slack: consumers
                    # are a block away except L9, which is ~4us later)
                    mask_mul("gps" if (not sl["fine"] and u == 1) else "dve",
                             dst, dst, m678[p_lo:112, 0, u * wsz:(u + 1) * wsz])
                del sl["zhs"]

            def emit_slot_l9(i):
                b = i - 2
                sl = slots[i]
                hm678 = sl["hr678"]
                fine = sl["fine"]
                osb = outp.tile([10, BLK], f32, tag="osb", bufs=2, name=f"osb_{b}")
                wins, wsz = slot_wins(fine)
                nmm = wsz // MSUB
                for u in wins:
                    z9 = psp.tile([128, wsz], f32, tag="ps", name=f"z9_{b}_{u}")
                    for t in range(nmm):
                        rhs_sl = slice(u * wsz + t * MSUB, u * wsz + (t + 1) * MSUB)
                        nc.tensor.matmul(z9[0:10, bass.ts(t, MSUB)],
                                         w789[96:112, 48:58],
                                         hm678[96:112, 0, rhs_sl],
                                         start=True, stop=True, tile_position=(96, 0))
                    if u % 2 == 0:
                        nc.scalar.activation(osb[:, u * wsz:(u + 1) * wsz],
                                             z9[0:10, :], IDENT, bias=b9[:, 0:1])
                    else:
                        nc.vector.tensor_scalar(osb[:, u * wsz:(u + 1) * wsz],
                                                z9[0:10, :], b9[:, 0:1], None, ADD)
                nc.sync.dma_start(out_d[:, bass.ts(b, BLK)], osb[:])
                if (i - 1) in slots:
                    del slots[i - 1]

            # --- schedule ---------------------------------------------------------
            # PE warmup: dummy matmuls on a memset scratch tile so the HAM clock
            # gate opens before real work arrives (weights/x still in DMA).
            nc.vector.memset(scratch[:], 0.0)
            zw = psp.tile([128, MSUB], f32, tag="ps", name="zwarm")
            for i in range(10):
                nc.tensor.matmul(zw[:], scratch[:, 0:128], scratch[:], start=True,
                                 stop=True)

            def warm(n):
                # filler matmuls that keep the PE HAM activity window busy while
                # a dependency chain stalls the real stream (tail)
                zf = psp.tile([128, MSUB], f32, tag="ps", name="zf")
                for _ in range(n):
                    nc.tensor.matmul(zf[:], scratch[:, 0:128], scratch[:],
                                     start=True, stop=True)

            issue_pack_dmas(0)
            emit_layer(0, 0)                       # L1(0) during startup
            for b in range(NBLK):
                if b + 1 < NBLK:
                    issue_pack_dmas(b + 1)
                if b == 2:
                    issue_m678_dma(4)              # phantom tail-slot planes
                if b == 3:
                    issue_m678_dma(5)
                emit_layer(b, 1)                   # L2
                if b >= 1:
                    emit_slot_trio(b - 1)          # L6(b-1)+L7(b-2)+L8(b-3)
                emit_layer(b, 2)                   # L3
                if b >= 1:
                    emit_slot_drainmask(b - 1)
                emit_layer(b, 3)                   # L4
                if b + 1 < NBLK:
                    emit_layer(b + 1, 0)           # L1(b+1) pipelined ahead
                emit_layer(b, 4)                   # L5
                hm5s[b] = hrs.pop((b, 4))
                if b >= 1 and b - 1 >= 2:
                    emit_slot_l9(b - 1)            # out block b-3; also covers the
                                                   # L5 drain latency at the tail
            # tail: remaining slots at fine (512) granularity so the serial chain
            # pipelines per-window. warm() filler is emitted BEFORE each
            # dependency-stalled group so it runs DURING the stall (engine FIFO)
            # and keeps the PE clock at 8/8.
            for i in range(NBLK - 1, NSLOT):
                warm(8)
                emit_slot_trio(i, fine=True)
                emit_slot_drainmask(i)
                warm(8)
                if i >= 2:
                    emit_slot_l9(i)

    _dedup_ldweights(nc)
    nc.compile()
    return nc


def _get_program():
    if "nc" not in _PROG:
        _PROG["nc"] = _build_program()
    return _PROG["nc"]


def _host_prep(inputs):
    """Build per-core input maps (numpy only)."""
    x = np.asarray(inputs["x"], dtype=np.float32)
    Ws = [np.asarray(inputs[f"W{i}"], dtype=np.float32) for i in range(1, 10)]
    bs = [np.asarray(inputs[f"b{i}"], dtype=np.float32) for i in range(1, 10)]
    ms = [np.asarray(inputs[f"m{i}"], dtype=np.float32) for i in range(1, 9)]

    # fold dropout scale into next layer's weights; binarize masks
    Wf = [Ws[0]]
    for i in range(1, 9):
        s = float(ms[i - 1].max())
        if s <= 0.0:  # degenerate all-dropped mask; keep weights unscaled
            s = 1.0
        Wf.append(Ws[i] * np.float32(s))

    # weight blob: w1@0 w2@256 w3@512 w4@1536 w5@2560 w6@2816 w789@2880
    WOFF = {1: 0, 2: 256, 3: 512, 4: 1536, 5: 2560, 6: 2816, 789: 2880}
    wb = np.zeros((128, 2944), dtype=np.float16)
    for l in range(1, 7):
        W = Wf[l - 1]
        K, N = W.shape
        for k in range((K + 127) // 128):
            blk = W[k * 128:(k + 1) * 128].astype(np.float16)
            wb[: blk.shape[0], WOFF[l] + k * N: WOFF[l] + k * N + N] = blk
    wb[0:64, 2880:2912] = Wf[6].astype(np.float16)    # W7
    wb[64:96, 2912:2928] = Wf[7].astype(np.float16)   # W8
    wb[96:112, 2928:2938] = Wf[8].astype(np.float16)  # W9
    wb1, wb = np.ascontiguousarray(wb[:, 0:256]), np.ascontiguousarray(wb[:, 256:])
    bb = np.zeros((128, 12), dtype=np.float32)
    bb[:, 0] = bs[0]
    bb[:, 1], bb[:, 2] = bs[1][0:128], bs[1][128:256]
    for c in range(4):
        bb[:, 3 + c] = bs[2][c * 128:(c + 1) * 128]
    bb[:, 7], bb[:, 8] = bs[3][0:128], bs[3][128:256]
    bb[:, 9] = bs[4]
    bb[0:64, 10], bb[64:96, 10], bb[96:112, 10] = bs[5], bs[6], bs[7]
    bb[0:10, 11] = bs[8]
    shared = {"WB1": wb1, "WB": wb, "BB": bb}

    in_maps = []
    for c in range(NCORES):
        sl = slice(c * SHARD, (c + 1) * SHARD)
        pack = np.zeros((NBLK, 128, NPACK, BLK), dtype=np.float16)
        m678 = np.zeros((NSLOT, 128, 1, BLK), dtype=np.float16)
        xT = x[sl].T  # (256, SHARD)
        mT = [None] + [(ms[i][sl] != 0).T.astype(np.float16) for i in range(8)]
        for b in range(NBLK):
            cs = slice(b * BLK, (b + 1) * BLK)
            pack[b, :, 0, :] = xT[0:128, cs]
            pack[b, :, 1, :] = xT[128:256, cs]
            pack[b, :, 2, :] = mT[1][:, cs]
            pack[b, :, 3, :], pack[b, :, 4, :] = mT[2][0:128, cs], mT[2][128:256, cs]
            for k in range(4):
                pack[b, :, 5 + k, :] = mT[3][k * 128:(k + 1) * 128, cs]
            pack[b, :, 9, :], pack[b, :, 10, :] = mT[4][0:128, cs], mT[4][128:256, cs]
            pack[b, :, 11, :] = mT[5][:, cs]
        # slot-staggered m678 planes: slot i masks {m6(i), m7(i-1), m8(i-2)}
        for i in range(NSLOT):
            if i < NBLK:
                m678[i, 0:64, 0, :] = mT[6][:, i * BLK:(i + 1) * BLK]
            if 0 <= i - 1 < NBLK:
                m678[i, 64:96, 0, :] = mT[7][:, (i - 1) * BLK:i * BLK]
            if 0 <= i - 2 < NBLK:
                m678[i, 96:112, 0, :] = mT[8][:, (i - 2) * BLK:(i - 1) * BLK]
        in_maps.append({"pack": pack, "M678": m678, **shared})
    return in_maps


def kernel(**inputs) -> np.ndarray:
    from concourse.bass_utils import run_bass_kernel_spmd

    nc = _get_program()
    in_maps = _host_prep(inputs)
    res = run_bass_kernel_spmd(nc, in_maps, list(range(NCORES)))
    out = np.empty((BATCH, DIMS[-1]), dtype=np.float32)
    for c in range(NCORES):
        out[c * SHARD:(c + 1) * SHARD, :] = res.results[c]["outT"].T
    return out


# revision 19
# speedup vs baseline: 1.1388x; 1.1388x over previous
"""Trainium2 Bass kernel for the 9-layer dense MLP (dropout-mask training forward).

Strategy (pure data parallel, 8 cores, 8192 batch rows each):
  - Activations kept transposed on-chip: features on partitions, batch cols on free dim.
    Each layer computes zT = W^T @ hT via nc.tensor.matmul(out, lhsT=W, rhs=hT).
  - fp16 weights/activations/masks (fp32 PSUM accumulation), fp32 biases + output.
  - Dropout masks binarized on host ({0,1} fp16); the 1/keep scale is folded into the
    next layer's weights.
  - Host pack layout [NBLK, 128, NPACK, BLK] so each per-block DMA is contiguous per
    partition (2 DMAs per block: x+m1, then the remaining masks).
  - PSUM: one shared pool of 4x [128,1024] fp32 tiles (8 banks). Matmuls write 512-col
    halves; drains are single FD=1024 instructions (fused bias+relu) split ~3:1
    ACT:DVE; mask multiplies are FD=2048 DVE tensor_tensor with a measured dose
    offloaded to GpSimd.
  - Small layers 6/7/8 partition-packed (offsets 0/64/96 via matmul tile_position);
    each ladder step drains immediately to SBUF so its PSUM tile recycles fast.
    Block b's ladder is software-pipelined into block b+1's big-layer bursts.
  - A short burst of dummy matmuls at t~1us keeps the PE HAM activity monitor busy so
    the array is at full clock (K=8/8) when real work arrives.
"""

import sys

sys.path.insert(0, "/opt/trn_rl_repo")

import numpy as np

DIMS = [256, 128, 256, 512, 256, 128, 64, 32, 16, 10]
NCORES = 8
BATCH = 65536
SHARD = BATCH // NCORES  # 8192
MSUB = 512               # matmul N (PSUM bank limit for fp32)
DSUB = 1024              # drain granularity (2 banks)
BLK = 2048               # block columns
NBLK = SHARD // BLK      # 4
NSUB = BLK // MSUB       # 4
NDR = BLK // DSUB        # 2

# pack chunk layout (each chunk = 128 partitions x BLK cols, fp16), per block:
#   0,1: xT   2: m1   3,4: m2   5-8: m3   9,10: m4   11: m5
# m678 ships separately as NBLK+2 slot-staggered planes (rows 0:64 = m6(slot),
# 64:96 = m7(slot-1), 96:112 = m8(slot-2)) to match the slot-fused ladder.
NPACK = 12
NSLOT = NBLK + 2

_PROG = {}


def _raise_sbuf_cap():
    # tile_utils.max_sbuf_usage is a stale 192KB constant; cayman has 208KB usable.
    import concourse.tile_utils as tu

    if getattr(tu, "max_sbuf_usage", 0) < 206 * 1024:
        tu.max_sbuf_usage = 206 * 1024


def _dedup_ldweights(nc):
    """Remove back-to-back redundant LDWEIGHTS (same stationary operand) so
    consecutive same-weight matmuls pipeline on the PE. Only drops LDW
    instructions that carry no semaphore waits/updates."""
    removed = 0
    for fn in nc.m.functions:
        for blk in fn.blocks:
            il = blk.instructions
            keep, last_sig = [], None
            for inst in il:
                nm = type(inst).__name__
                if nm == "InstLdweights":
                    sig = (str(inst.ins[0]), str(inst.is_transpose), str(inst.perf_mode),
                           str(getattr(inst, "tile_position", None)))
                    si = inst.sync_info
                    clean = si is None or (not si.on_wait and not si.on_update)
                    if sig == last_sig and clean:
                        removed += 1
                        continue
                    last_sig = sig
                keep.append(inst)
            if removed and len(keep) != len(il):
                while il:
                    il.pop()
                il.extend(keep)
    return removed


def _build_program():
    import concourse.bass as bass
    import concourse.tile as tile
    from concourse import bacc, mybir

    _raise_sbuf_cap()

    f16 = mybir.dt.float16
    f32 = mybir.dt.float32
    RELU = mybir.ActivationFunctionType.Relu
    IDENT = mybir.ActivationFunctionType.Identity
    ADD = mybir.AluOpType.add
    MAX = mybir.AluOpType.max

    nc = bacc.Bacc("TRN2", target_bir_lowering=False, debug=False, num_devices=NCORES)

    pack_d = nc.dram_tensor("pack", [NBLK, 128, NPACK, BLK], f16, kind="ExternalInput").ap()
    m678_d = nc.dram_tensor("M678", [NSLOT, 128, 1, BLK], f16, kind="ExternalInput").ap()
    # weights in two host-laid-out fp16 blobs (W1 separate so the first
    # LDWEIGHTS isn't gated on the full blob), biases in one fp32 blob
    wb1_d = nc.dram_tensor("WB1", [128, 256], f16, kind="ExternalInput").ap()
    wb_d = nc.dram_tensor("WB", [128, 2688], f16, kind="ExternalInput").ap()
    bb_d = nc.dram_tensor("BB", [128, 12], f32, kind="ExternalInput").ap()
    out_d = nc.dram_tensor("outT", [10, SHARD], f32, kind="ExternalOutput").ap()

    with tile.TileContext(nc) as tc:
        with (
            tc.tile_pool(name="wpool", bufs=1) as wp,
            tc.tile_pool(name="mk", bufs=2) as mkp,
            tc.tile_pool(name="hr", bufs=1) as hrp,
            tc.tile_pool(name="osb", bufs=2) as outp,
            tc.tile_pool(name="ps", bufs=4, space="PSUM") as psp,
        ):
            wall = wp.tile([128, 2944], f16, tag="wall")
            ball = wp.tile([128, 12], f32, tag="ball")
            scratch = wp.tile([128, 512], f16, tag="scratch")
            # blob column offsets: w1@0(256) w2@256(256) w3@512(1024) w4@1536(1024)
            #   w5@2560(256) w6@2816(64) w789@2880(64: W7 r0-63 c0-31, W8 r64-95
            #   c32-47, W9 r96-111 c48-57)
            WOFF = {1: 0, 2: 256, 3: 512, 4: 1536, 5: 2560, 6: 2816, 789: 2880}
            w789 = wall[:, WOFF[789]:WOFF[789] + 64]
            b15 = ball[:, 0:10]
            b678 = ball[:, 10:11]
            b9 = ball[0:10, 11:12]

            def wslice(l, k, c, N):
                base = WOFF[l] + k * N
                return wall[:, base + c * 128: base + (c + 1) * 128]

            def drain_relu(eng, dst, zsrc, bias_ap):
                if eng == "act":
                    nc.scalar.activation(dst, zsrc, RELU, bias=bias_ap)
                else:
                    nc.vector.tensor_scalar(dst, zsrc, bias_ap, 0.0, ADD, MAX)

            def mask_mul(eng, dst, src, msrc):
                if eng == "gps":
                    nc.gpsimd.tensor_mul(dst, src, msrc)
                else:
                    nc.vector.tensor_mul(dst, src, msrc)

            # drain engine picker: ~70:30 act:dve (ACT is cheaper per element but
            # DVE has mask work too; this balances their queues)
            dr_i = [0]

            def pick_drain():
                i = dr_i[0]
                dr_i[0] += 1
                return "dve" if i % 10 in (2, 5, 8) else "act"

            state = {}
            packs = {}
            hrs = {}

            def issue_pack_dmas(b):
                # per-chunk tiles/DMAs: each mask tile's ring slot is released as
                # soon as its own layer consumes it, so block b+2's DMAs start
                # early instead of waiting for ALL of block b's masks (WAR).
                pk3 = mkp.tile([128, 3, BLK], f16, tag="pk3", name=f"pk3_{b}")
                m2t = mkp.tile([128, 2, BLK], f16, tag="m2", name=f"m2_{b}")
                m3t = mkp.tile([128, 4, BLK], f16, tag="m3", name=f"m3_{b}")
                m4t = mkp.tile([128, 2, BLK], f16, tag="m4", name=f"m4_{b}")
                m5t = mkp.tile([128, 1, BLK], f16, tag="m5", name=f"m5_{b}")
                if b == 0:
                    nc.sync.dma_start(wall[:, 0:256], wb1_d[:])
                    nc.sync.dma_start(ball[:], bb_d[:])
                    nc.sync.dma_start(pk3[:, :, 0:DSUB], pack_d[0, :, 0:3, 0:DSUB])
                    nc.sync.dma_start(pk3[:, :, DSUB:BLK], pack_d[0, :, 0:3, DSUB:BLK])
                    nc.sync.dma_start(wall[:, 256:], wb_d[:])
                else:
                    nc.sync.dma_start(pk3[:], pack_d[b, :, 0:3, :])
                nc.sync.dma_start(m2t[:], pack_d[b, :, 3:5, :])
                nc.sync.dma_start(m3t[:], pack_d[b, :, 5:9, :])
                nc.sync.dma_start(m4t[:], pack_d[b, :, 9:11, :])
                nc.sync.dma_start(m5t[:], pack_d[b, :, 11:12, :])
                packs[b] = (pk3, m2t, m3t, m4t, m5t)
                issue_m678_dma(b)

            m678s = {}

            def issue_m678_dma(i):
                m678s[i] = mkp.tile([128, 1, BLK], f16, tag="m678", name=f"m678s_{i}",
                                    bufs=3)
                nc.sync.dma_start(m678s[i][:], m678_d[i])

            # (Kc, layer, wN, Cc, bias_off, hrtag)
            LAYER_CFG = [
                (2, 1, 128, 1, 0, "hr1"),
                (1, 2, 256, 2, 1, "hr2"),
                (2, 3, 512, 4, 3, "hr3"),
                (4, 4, 256, 2, 7, "hr4"),
                (2, 5, 128, 1, 9, "hr5"),
            ]
            # mask engine schedule per layer index. GpSimd is ~4x slower than DVE
            # per element, so it only gets masks with slack before their consumer:
            # m1 (L1 runs a block ahead) and m5 (ladder consumes it next block).
            # L2/L3/L4 masks sit on the next layer's critical path -> DVE only.
            MASK_MODE = {
                0: ["dve2"],
                1: ["dve", "dve"],
                2: ["dve", "dve", "dve", "dve"],
                3: ["dve", "dve"],
                4: ["dve2"],
            }
            # per-layer drain engine assignment, in (u, c) order. ACT-heavy for
            # the mid-block bulk (L3/L4); the boundary-critical L2 keeps one on
            # DVE so its chain isn't stuck behind the ACT queue.
            DRAIN_ENG = {
                0: ["dve", "act"],
                1: ["act", "act", "dve", "act"],
                2: ["act", "act", "act", "dve", "act", "act", "act", "act"],
                3: ["act", "act", "act", "act"],
                4: ["dve", "act"],
            }

            def emit_layer(b, li):
                Kc, wl, wN, Cc, boff, hrtag = LAYER_CFG[li]
                hr = hrp.tile([128, Cc, BLK], f16, tag=hrtag, name=hrtag + f"_{b}",
                              bufs=2 if hrtag in ("hr5", "hr2", "hr1") else 1)
                pk3 = packs[b][0]
                hin = pk3 if li == 0 else hrs[(b, li - 1)]

                def msl(c, cols):
                    if li == 0:
                        return pk3[:, 2, cols]
                    return packs[b][li][:, c, cols]

                zs = {}
                for c in range(Cc):
                    for u in range(NDR):
                        zs[c, u] = psp.tile([128, DSUB], f32, tag="ps",
                                            name=f"z_{hrtag}_{b}_{c}_{u}")
                if b == 0 and li == 0:
                    # startup: u-outer so each 1024-col window flows MM -> drain
                    # -> mask as early as possible
                    for u in range(NDR):
                        for k in range(Kc):
                            for t in range(2):
                                nc.tensor.matmul(
                                    zs[0, u][:, bass.ts(t, MSUB)],
                                    wslice(wl, k, 0, wN),
                                    hin[:, k, u * DSUB + t * MSUB:
                                        u * DSUB + (t + 1) * MSUB],
                                    start=(k == 0), stop=(k == Kc - 1))
                        drain_relu("dve" if u == 0 else "act",
                                   hr[:, 0, bass.ts(u, DSUB)], zs[0, u][:],
                                   b15[:, 0:1])
                        mask_mul("dve", hr[:, 0, bass.ts(u, DSUB)],
                                 hr[:, 0, bass.ts(u, DSUB)],
                                 msl(0, bass.ts(u, DSUB)))
                    hrs[(b, li)] = hr
                    return
                # weight-major matmuls so consecutive MMs share one LDWEIGHTS
                for c in range(Cc):
                    for k in range(Kc):
                        wap = wslice(wl, k, c, wN)
                        for t in range(NSUB):
                            nc.tensor.matmul(
                                zs[c, t // 2][:, bass.ts(t % 2, MSUB)], wap,
                                hin[:, k, bass.ts(t, MSUB)],
                                start=(k == 0), stop=(k == Kc - 1))
                de = DRAIN_ENG[li]
                for u in range(NDR):
                    for c in range(Cc):
                        drain_relu(de[u * Cc + c], hr[:, c, bass.ts(u, DSUB)],
                                   zs[c, u][:], b15[:, boff + c:boff + c + 1])
                full = slice(0, BLK)
                for c in range(Cc):
                    mode = MASK_MODE[li][c]
                    if mode == "dve":
                        mask_mul("dve", hr[:, c, full], hr[:, c, full], msl(c, full))
                    elif mode == "dve2":
                        for uu in range(NDR):
                            hs = bass.ts(uu, DSUB)
                            mask_mul("dve", hr[:, c, hs], hr[:, c, hs], msl(c, hs))
                    else:
                        hs0, hs1 = bass.ts(0, DSUB), bass.ts(1, DSUB)
                        mask_mul("dve", hr[:, c, hs0], hr[:, c, hs0], msl(c, hs0))
                        mask_mul("gps", hr[:, c, hs1], hr[:, c, hs1], msl(c, hs1))
                hrs[(b, li)] = hr
                if li > 0:
                    del hrs[(b, li - 1)]

            # --- small-layer ladder, slot-fused across blocks --------------------
            # Slot i co-issues three INDEPENDENT small matmuls from staggered
            # blocks into disjoint partition ranges of ONE PSUM tile:
            #   rows 0:64   L6(block i)    cols 0:64   of the PE array
            #   rows 64:96  L7(block i-1)  cols 64:96  (tile_position (0,64))
            #   rows 96:112 L8(block i-2)  cols 96:128 (tile_position (64,96))
            # They run concurrently (disjoint subarrays), and ONE drain + ONE
            # mask serves all three (the host staggers the m678 mask planes the
            # same way). L9(block i-2) then reads rows 96:112 after the mask.
            slots = {}
            hm5s = {}

            def slot_wins(fine):
                return (range(NSUB), MSUB) if fine else (range(NDR), DSUB)

            def emit_slot_trio(i, fine=False):
                sl = slots.setdefault(i, {})
                sl["hr678"] = hrp.tile([128, 1, BLK], f16, tag="hr678",
                                       name=f"hr678_{i}", bufs=2)
                prev = slots.get(i - 1)
                wins, wsz = slot_wins(fine)
                nmm = wsz // MSUB
                zhs = {}
                for u in wins:
                    zhs[u] = psp.tile([128, wsz], f32, tag="ps", name=f"zh_{i}_{u}")
                sl["zhs"] = zhs
                sl["fine"] = fine
                parts = []
                if i <= NBLK - 1:
                    parts.append((0, 64, wall[:, WOFF[6]:WOFF[6] + 64], None,
                                  lambda cs: hm5s[i][:, 0, cs]))
                if 1 <= i <= NBLK:
                    parts.append((64, 96, w789[0:64, 0:32], (0, 64),
                                  lambda cs: prev["hr678"][0:64, 0, cs]))
                if 2 <= i <= NBLK + 1:
                    parts.append((96, 112, w789[64:96, 32:48], (64, 96),
                                  lambda cs: prev["hr678"][64:96, 0, cs]))
                sl["p_lo"] = parts[0][0]
                # weight-major: one LDW per weight per slot (consecutive dedup),
                # streams of different col-groups overlap on the PE
                for p0, p1, wap, tpos, rhs_of in parts:
                    for u in wins:
                        for t in range(nmm):
                            cs = slice(u * wsz + t * MSUB, u * wsz + (t + 1) * MSUB)
                            out_sl = zhs[u][p0:p1, bass.ts(t, MSUB)]
                            if tpos is None:
                                nc.tensor.matmul(out_sl, wap, rhs_of(cs),
                                                 start=True, stop=True)
                            else:
                                nc.tensor.matmul(out_sl, wap, rhs_of(cs),
                                                 start=True, stop=True,
                                                 tile_position=tpos)
                if i in hm5s:
                    del hm5s[i]

            def emit_slot_drainmask(i):
                sl = slots[i]
                p_lo = sl["p_lo"]
                hr678 = sl["hr678"]
                m678 = m678s[i]
                wins, wsz = slot_wins(sl["fine"])
                for u in wins:
                    dst = hr678[p_lo:112, 0, u * wsz:(u + 1) * wsz]
                    drain_relu("dve" if u % 2 == 0 else "act", dst,
                               sl["zhs"][u][p_lo:112, :], b678[p_lo:112, 0:1])
                    # hs1 half -> gps only at coarse granularity (slack: consumers
                    # are a block away except L9, which is ~4us later)
                    mask_mul("gps" if (not sl["fine"] and u == 1) else "dve",
                             dst, dst, m678[p_lo:112, 0, u * wsz:(u + 1) * wsz])
                del sl["zhs"]

            def emit_slot_l9(i):
                b = i - 2
                sl = slots[i]
                hm678 = sl["hr678"]
                fine = sl["fine"]
                osb = outp.tile([10, BLK], f32, tag="osb", bufs=2, name=f"osb_{b}")
                wins, wsz = slot_wins(fine)
                nmm = wsz // MSUB
                for u in wins:
                    z9 = psp.tile([128, wsz], f32, tag="ps", name=f"z9_{b}_{u}")
                    for t in range(nmm):
                        rhs_sl = slice(u * wsz + t * MSUB, u * wsz + (t + 1) * MSUB)
                        nc.tensor.matmul(z9[0:10, bass.ts(t, MSUB)],
                                         w789[96:112, 48:58],
                                         hm678[96:112, 0, rhs_sl],
                                         start=True, stop=True, tile_position=(96, 0))
                    if u % 2 == 0:
                        nc.scalar.activation(osb[:, u * wsz:(u + 1) * wsz],
                                             z9[0:10, :], IDENT, bias=b9[:, 0:1])
                    else:
                        nc.vector.tensor_scalar(osb[:, u * wsz:(u + 1) * wsz],
                                                z9[0:10, :], b9[:, 0:1], None, ADD)
                nc.sync.dma_start(out_d[:, bass.ts(b, BLK)], osb[:])
                if (i - 1) in slots:
                    del slots[i - 1]

            # --- schedule ---------------------------------------------------------
            # PE warmup: dummy matmuls on a memset scratch tile so the HAM clock
            # gate opens before real work arrives (weights/x still in DMA).
            nc.vector.memset(scratch[:], 0.0)
            zw = psp.tile([128, MSUB], f32, tag="ps", name="zwarm")
            for i in range(10):
                nc.tensor.matmul(zw[:], scratch[:, 0:128], scratch[:], start=True,
                                 stop=True)

            def warm(n):
                # filler matmuls that keep the PE HAM activity window busy while
                # a dependency chain stalls the real stream (tail)
                zf = psp.tile([128, MSUB], f32, tag="ps", name="zf")
                for _ in range(n):
                    nc.tensor.matmul(zf[:], scratch[:, 0:128], scratch[:],
                                     start=True, stop=True)

            issue_pack_dmas(0)
            emit_layer(0, 0)                       # L1(0) during startup
            for b in range(NBLK):
                if b + 1 < NBLK:
                    issue_pack_dmas(b + 1)
                if b == 2:
                    issue_m678_dma(4)              # phantom tail-slot planes
                if b == 3:
                    issue_m678_dma(5)
                if b >= 1:
                    warm(3)                        # bridge the block handover
                emit_layer(b, 1)                   # L2
                if b >= 1:
                    emit_slot_trio(b - 1)          # L6(b-1)+L7(b-2)+L8(b-3)
                emit_layer(b, 2)                   # L3
                if b >= 1:
                    emit_slot_drainmask(b - 1)
                emit_layer(b, 3)                   # L4
                if b + 1 < NBLK:
                    emit_layer(b + 1, 0)           # L1(b+1) pipelined ahead
                if b >= 1 and b - 1 >= 2:
                    emit_slot_l9(b - 1)            # out block b-3
                emit_layer(b, 4)                   # L5
                hm5s[b] = hrs.pop((b, 4))

            # tail: remaining slots at fine (512) granularity so the serial chain
            # pipelines per-window; small warm() fillers run during the chain
            # stalls (engine FIFO) and keep the PE clock at 8/8
            warm(4)
            for i in range(NBLK - 1, NSLOT):
                emit_slot_trio(i, fine=True)
                warm(4)
                emit_slot_drainmask(i)
                if i >= 2:
                    emit_slot_l9(i)
                warm(4)

    _dedup_ldweights(nc)
    nc.compile()
    return nc


def _get_program():
    if "nc" not in _PROG:
        _PROG["nc"] = _build_program()
    return _PROG["nc"]


def _host_prep(inputs):
    """Build per-core input maps (numpy only)."""
    x = np.asarray(inputs["x"], dtype=np.float32)
    Ws = [np.asarray(inputs[f"W{i}"], dtype=np.float32) for i in range(1, 10)]
    bs = [np.asarray(inputs[f"b{i}"], dtype=np.float32) for i in range(1, 10)]
    ms = [np.asarray(inputs[f"m{i}"], dtype=np.float32) for i in range(1, 9)]

    # fold dropout scale into next layer's weights; binarize masks
    Wf = [Ws[0]]
    for i in range(1, 9):
        s = float(ms[i - 1].max())
        if s <= 0.0:  # degenerate all-dropped mask; keep weights unscaled
            s = 1.0
        Wf.append(Ws[i] * np.float32(s))

    # weight blob: w1@0 w2@256 w3@512 w4@1536 w5@2560 w6@2816 w789@2880
    WOFF = {1: 0, 2: 256, 3: 512, 4: 1536, 5: 2560, 6: 2816, 789: 2880}
    wb = np.zeros((128, 2944), dtype=np.float16)
    for l in range(1, 7):
        W = Wf[l - 1]
        K, N = W.shape
        for k in range((K + 127) // 128):
            blk = W[k * 128:(k + 1) * 128].astype(np.float16)
            wb[: blk.shape[0], WOFF[l] + k * N: WOFF[l] + k * N + N] = blk
    wb[0:64, 2880:2912] = Wf[6].astype(np.float16)    # W7
    wb[64:96, 2912:2928] = Wf[7].astype(np.float16)   # W8
    wb[96:112, 2928:2938] = Wf[8].astype(np.float16)  # W9
    wb1, wb = np.ascontiguousarray(wb[:, 0:256]), np.ascontiguousarray(wb[:, 256:])
    bb = np.zeros((128, 12), dtype=np.float32)
    bb[:, 0] = bs[0]
    bb[:, 1], bb[:, 2] = bs[1][0:128], bs[1][128:256]
    for c in range(4):
        bb[:, 3 + c] = bs[2][c * 128:(c + 1) * 128]
    bb[:, 7], bb[:, 8] = bs[3][0:128], bs[3][128:256]
    bb[:, 9] = bs[4]
    bb[0:64, 10], bb[64:96, 10], bb[96:112, 10] = bs[5], bs[6], bs[7]
    bb[0:10, 11] = bs[8]
    shared = {"WB1": wb1, "WB": wb, "BB": bb}

    in_maps = []
    for c in range(NCORES):
        sl = slice(c * SHARD, (c + 1) * SHARD)
        pack = np.zeros((NBLK, 128, NPACK, BLK), dtype=np.float16)
        m678 = np.zeros((NSLOT, 128, 1, BLK), dtype=np.float16)
        xT = x[sl].T  # (256, SHARD)
        mT = [None] + [(ms[i][sl] != 0).T.astype(np.float16) for i in range(8)]
        for b in range(NBLK):
            cs = slice(b * BLK, (b + 1) * BLK)
            pack[b, :, 0, :] = xT[0:128, cs]
            pack[b, :, 1, :] = xT[128:256, cs]
            pack[b, :, 2, :] = mT[1][:, cs]
            pack[b, :, 3, :], pack[b, :, 4, :] = mT[2][0:128, cs], mT[2][128:256, cs]
            for k in range(4):
                pack[b, :, 5 + k, :] = mT[3][k * 128:(k + 1) * 128, cs]
            pack[b, :, 9, :], pack[b, :, 10, :] = mT[4][0:128, cs], mT[4][128:256, cs]
            pack[b, :, 11, :] = mT[5][:, cs]
        # slot-staggered m678 planes: slot i masks {m6(i), m7(i-1), m8(i-2)}
        for i in range(NSLOT):
            if i < NBLK:
                m678[i, 0:64, 0, :] = mT[6][:, i * BLK:(i + 1) * BLK]
            if 0 <= i - 1 < NBLK:
                m678[i, 64:96, 0, :] = mT[7][:, (i - 1) * BLK:i * BLK]
            if 0 <= i - 2 < NBLK:
                m678[i, 96:112, 0, :] = mT[8][:, (i - 2) * BLK:(i - 1) * BLK]
        in_maps.append({"pack": pack, "M678": m678, **shared})
    return in_maps


def kernel(**inputs) -> np.ndarray:
    from concourse.bass_utils import run_bass_kernel_spmd

    nc = _get_program()
    in_maps = _host_prep(inputs)
    res = run_bass_kernel_spmd(nc, in_maps, list(range(NCORES)))
    out = np.empty((BATCH, DIMS[-1]), dtype=np.float32)
    for c in range(NCORES):
        out[c * SHARD:(c + 1) * SHARD, :] = res.results[c]["outT"].T
    return out


# revision 21
# speedup vs baseline: 1.1538x; 1.0132x over previous
"""Trainium2 Bass kernel for the 9-layer dense MLP (dropout-mask training forward).

Strategy (pure data parallel, 8 cores, 8192 batch rows each):
  - Activations kept transposed on-chip: features on partitions, batch cols on free dim.
    Each layer computes zT = W^T @ hT via nc.tensor.matmul(out, lhsT=W, rhs=hT).
  - fp16 weights/activations/masks (fp32 PSUM accumulation), fp32 biases + output.
  - Dropout masks binarized on host ({0,1} fp16); the 1/keep scale is folded into the
    next layer's weights.
  - Host pack layout [NBLK, 128, NPACK, BLK] so each per-block DMA is contiguous per
    partition (2 DMAs per block: x+m1, then the remaining masks).
  - PSUM: one shared pool of 4x [128,1024] fp32 tiles (8 banks). Matmuls write 512-col
    halves; drains are single FD=1024 instructions (fused bias+relu) split ~3:1
    ACT:DVE; mask multiplies are FD=2048 DVE tensor_tensor with a measured dose
    offloaded to GpSimd.
  - Small layers 6/7/8 partition-packed (offsets 0/64/96 via matmul tile_position);
    each ladder step drains immediately to SBUF so its PSUM tile recycles fast.
    Block b's ladder is software-pipelined into block b+1's big-layer bursts.
  - A short burst of dummy matmuls at t~1us keeps the PE HAM activity monitor busy so
    the array is at full clock (K=8/8) when real work arrives.
"""

import sys

sys.path.insert(0, "/opt/trn_rl_repo")

import numpy as np

DIMS = [256, 128, 256, 512, 256, 128, 64, 32, 16, 10]
NCORES = 8
BATCH = 65536
SHARD = BATCH // NCORES  # 8192
MSUB = 512               # matmul N (PSUM bank limit for fp32)
DSUB = 1024              # drain granularity (2 banks)
BLK = 2048               # block columns
NBLK = SHARD // BLK      # 4
NSUB = BLK // MSUB       # 4
NDR = BLK // DSUB        # 2

# pack chunk layout (each chunk = 128 partitions x BLK cols, fp16), per block:
#   0,1: xT   2: m1   3,4: m2   5-8: m3   9,10: m4   11: m5
# m678 ships separately as NBLK+2 slot-staggered planes (rows 0:64 = m6(slot),
# 64:96 = m7(slot-1), 96:112 = m8(slot-2)) to match the slot-fused ladder.
NPACK = 12
NSLOT = NBLK + 2

_PROG = {}


def _raise_sbuf_cap():
    # tile_utils.max_sbuf_usage is a stale 192KB constant; cayman has 208KB usable.
    import concourse.tile_utils as tu

    if getattr(tu, "max_sbuf_usage", 0) < 206 * 1024:
        tu.max_sbuf_usage = 206 * 1024


def _dedup_ldweights(nc):
    """Remove back-to-back redundant LDWEIGHTS (same stationary operand) so
    consecutive same-weight matmuls pipeline on the PE. Only drops LDW
    instructions that carry no semaphore waits/updates."""
    removed = 0
    for fn in nc.m.functions:
        for blk in fn.blocks:
            il = blk.instructions
            keep, last_sig = [], None
            for inst in il:
                nm = type(inst).__name__
                if nm == "InstLdweights":
                    sig = (str(inst.ins[0]), str(inst.is_transpose), str(inst.perf_mode),
                           str(getattr(inst, "tile_position", None)))
                    si = inst.sync_info
                    clean = si is None or (not si.on_wait and not si.on_update)
                    if sig == last_sig and clean:
                        removed += 1
                        continue
                    last_sig = sig
                keep.append(inst)
            if removed and len(keep) != len(il):
                while il:
                    il.pop()
                il.extend(keep)
    return removed


def _build_program():
    import concourse.bass as bass
    import concourse.tile as tile
    from concourse import bacc, mybir

    _raise_sbuf_cap()

    f16 = mybir.dt.float16
    f32 = mybir.dt.float32
    RELU = mybir.ActivationFunctionType.Relu
    IDENT = mybir.ActivationFunctionType.Identity
    ADD = mybir.AluOpType.add
    MAX = mybir.AluOpType.max

    nc = bacc.Bacc("TRN2", target_bir_lowering=False, debug=False, num_devices=NCORES)

    pack_d = nc.dram_tensor("pack", [NBLK, 128, NPACK, BLK], f16, kind="ExternalInput").ap()
    m678_d = nc.dram_tensor("M678", [NSLOT, 128, 1, BLK], f16, kind="ExternalInput").ap()
    # weights in two host-laid-out fp16 blobs (W1 separate so the first
    # LDWEIGHTS isn't gated on the full blob), biases in one fp32 blob
    wb1_d = nc.dram_tensor("WB1", [128, 256], f16, kind="ExternalInput").ap()
    wb_d = nc.dram_tensor("WB", [128, 2688], f16, kind="ExternalInput").ap()
    bb_d = nc.dram_tensor("BB", [128, 12], f32, kind="ExternalInput").ap()
    out_d = nc.dram_tensor("outT", [10, SHARD], f32, kind="ExternalOutput").ap()

    with tile.TileContext(nc) as tc:
        with (
            tc.tile_pool(name="wpool", bufs=1) as wp,
            tc.tile_pool(name="mk", bufs=2) as mkp,
            tc.tile_pool(name="hr", bufs=1) as hrp,
            tc.tile_pool(name="osb", bufs=2) as outp,
            tc.tile_pool(name="ps", bufs=4, space="PSUM") as psp,
        ):
            wall = wp.tile([128, 2944], f16, tag="wall")
            ball = wp.tile([128, 12], f32, tag="ball")
            scratch = wp.tile([128, 512], f16, tag="scratch")
            # blob column offsets: w1@0(256) w2@256(256) w3@512(1024) w4@1536(1024)
            #   w5@2560(256) w6@2816(64) w789@2880(64: W7 r0-63 c0-31, W8 r64-95
            #   c32-47, W9 r96-111 c48-57)
            WOFF = {1: 0, 2: 256, 3: 512, 4: 1536, 5: 2560, 6: 2816, 789: 2880}
            w789 = wall[:, WOFF[789]:WOFF[789] + 64]
            b15 = ball[:, 0:10]
            b678 = ball[:, 10:11]
            b9 = ball[0:10, 11:12]

            def wslice(l, k, c, N):
                base = WOFF[l] + k * N
                return wall[:, base + c * 128: base + (c + 1) * 128]

            def drain_relu(eng, dst, zsrc, bias_ap):
                if eng == "act":
                    nc.scalar.activation(dst, zsrc, RELU, bias=bias_ap)
                else:
                    nc.vector.tensor_scalar(dst, zsrc, bias_ap, 0.0, ADD, MAX)

            def mask_mul(eng, dst, src, msrc):
                if eng == "gps":
                    nc.gpsimd.tensor_mul(dst, src, msrc)
                else:
                    nc.vector.tensor_mul(dst, src, msrc)

            # drain engine picker: ~70:30 act:dve (ACT is cheaper per element but
            # DVE has mask work too; this balances their queues)
            dr_i = [0]

            def pick_drain():
                i = dr_i[0]
                dr_i[0] += 1
                return "dve" if i % 10 in (2, 5, 8) else "act"

            state = {}
            packs = {}
            hrs = {}

            def issue_pack_dmas(b):
                # per-chunk tiles/DMAs: each mask tile's ring slot is released as
                # soon as its own layer consumes it, so block b+2's DMAs start
                # early instead of waiting for ALL of block b's masks (WAR).
                pk3 = mkp.tile([128, 3, BLK], f16, tag="pk3", name=f"pk3_{b}")
                m2t = mkp.tile([128, 2, BLK], f16, tag="m2", name=f"m2_{b}")
                m3t = mkp.tile([128, 4, BLK], f16, tag="m3", name=f"m3_{b}")
                m4t = mkp.tile([128, 2, BLK], f16, tag="m4", name=f"m4_{b}")
                m5t = mkp.tile([128, 1, BLK], f16, tag="m5", name=f"m5_{b}")
                if b == 0:
                    nc.sync.dma_start(wall[:, 0:256], wb1_d[:])
                    nc.sync.dma_start(ball[:], bb_d[:])
                    nc.sync.dma_start(pk3[:, :, 0:DSUB], pack_d[0, :, 0:3, 0:DSUB])
                    nc.sync.dma_start(pk3[:, :, DSUB:BLK], pack_d[0, :, 0:3, DSUB:BLK])
                    nc.sync.dma_start(wall[:, 256:], wb_d[:])
                else:
                    nc.sync.dma_start(pk3[:], pack_d[b, :, 0:3, :])
                nc.sync.dma_start(m2t[:], pack_d[b, :, 3:5, :])
                nc.sync.dma_start(m3t[:], pack_d[b, :, 5:9, :])
                nc.sync.dma_start(m4t[:], pack_d[b, :, 9:11, :])
                nc.sync.dma_start(m5t[:], pack_d[b, :, 11:12, :])
                packs[b] = (pk3, m2t, m3t, m4t, m5t)
                issue_m678_dma(b)

            m678s = {}

            def issue_m678_dma(i):
                m678s[i] = mkp.tile([128, 1, BLK], f16, tag="m678", name=f"m678s_{i}",
                                    bufs=3)
                nc.sync.dma_start(m678s[i][:], m678_d[i])

            # (Kc, layer, wN, Cc, bias_off, hrtag)
            LAYER_CFG = [
                (2, 1, 128, 1, 0, "hr1"),
                (1, 2, 256, 2, 1, "hr2"),
                (2, 3, 512, 4, 3, "hr3"),
                (4, 4, 256, 2, 7, "hr4"),
                (2, 5, 128, 1, 9, "hr5"),
            ]
            # mask engine schedule per layer index. GpSimd is ~4x slower than DVE
            # per element, so it only gets masks with slack before their consumer:
            # m1 (L1 runs a block ahead) and m5 (ladder consumes it next block).
            # L2/L3/L4 masks sit on the next layer's critical path -> DVE only.
            MASK_MODE = {
                0: ["dve2"],
                1: ["dve", "dve"],
                2: ["dve", "dve", "dve", "dve"],
                3: ["dve", "dve"],
                4: ["dve2"],
            }
            # per-layer drain engine assignment, in (u, c) order. ACT-heavy for
            # the mid-block bulk (L3/L4); the boundary-critical L2 keeps one on
            # DVE so its chain isn't stuck behind the ACT queue.
            DRAIN_ENG = {
                0: ["dve", "act"],
                1: ["act", "act", "dve", "act"],
                2: ["act", "act", "act", "dve", "act", "act", "act", "act"],
                3: ["act", "act", "act", "act"],
                4: ["dve", "act"],
            }

            def emit_layer(b, li):
                Kc, wl, wN, Cc, boff, hrtag = LAYER_CFG[li]
                hr = hrp.tile([128, Cc, BLK], f16, tag=hrtag, name=hrtag + f"_{b}",
                              bufs=2 if hrtag in ("hr5", "hr2", "hr1") else 1)
                pk3 = packs[b][0]
                hin = pk3 if li == 0 else hrs[(b, li - 1)]

                def msl(c, cols):
                    if li == 0:
                        return pk3[:, 2, cols]
                    return packs[b][li][:, c, cols]

                zs = {}
                for c in range(Cc):
                    for u in range(NDR):
                        zs[c, u] = psp.tile([128, DSUB], f32, tag="ps",
                                            name=f"z_{hrtag}_{b}_{c}_{u}")
                if b == 0 and li == 0:
                    # startup: u-outer so each 1024-col window flows MM -> drain
                    # -> mask as early as possible
                    for u in range(NDR):
                        for k in range(Kc):
                            for t in range(2):
                                nc.tensor.matmul(
                                    zs[0, u][:, bass.ts(t, MSUB)],
                                    wslice(wl, k, 0, wN),
                                    hin[:, k, u * DSUB + t * MSUB:
                                        u * DSUB + (t + 1) * MSUB],
                                    start=(k == 0), stop=(k == Kc - 1))
                        drain_relu("dve" if u == 0 else "act",
                                   hr[:, 0, bass.ts(u, DSUB)], zs[0, u][:],
                                   b15[:, 0:1])
                        mask_mul("dve", hr[:, 0, bass.ts(u, DSUB)],
                                 hr[:, 0, bass.ts(u, DSUB)],
                                 msl(0, bass.ts(u, DSUB)))
                    hrs[(b, li)] = hr
                    return
                # weight-major matmuls so consecutive MMs share one LDWEIGHTS
                for c in range(Cc):
                    for k in range(Kc):
                        wap = wslice(wl, k, c, wN)
                        for t in range(NSUB):
                            nc.tensor.matmul(
                                zs[c, t // 2][:, bass.ts(t % 2, MSUB)], wap,
                                hin[:, k, bass.ts(t, MSUB)],
                                start=(k == 0), stop=(k == Kc - 1))
                de = DRAIN_ENG[li]
                for u in range(NDR):
                    for c in range(Cc):
                        drain_relu(de[u * Cc + c], hr[:, c, bass.ts(u, DSUB)],
                                   zs[c, u][:], b15[:, boff + c:boff + c + 1])
                full = slice(0, BLK)
                for c in range(Cc):
                    mode = MASK_MODE[li][c]
                    if mode == "dve":
                        mask_mul("dve", hr[:, c, full], hr[:, c, full], msl(c, full))
                    elif mode == "dve2":
                        for uu in range(NDR):
                            hs = bass.ts(uu, DSUB)
                            mask_mul("dve", hr[:, c, hs], hr[:, c, hs], msl(c, hs))
                    else:
                        hs0, hs1 = bass.ts(0, DSUB), bass.ts(1, DSUB)
                        mask_mul("dve", hr[:, c, hs0], hr[:, c, hs0], msl(c, hs0))
                        mask_mul("gps", hr[:, c, hs1], hr[:, c, hs1], msl(c, hs1))
                hrs[(b, li)] = hr
                if li > 0:
                    del hrs[(b, li - 1)]

            # --- small-layer ladder, slot-fused across blocks --------------------
            # Slot i co-issues three INDEPENDENT small matmuls from staggered
            # blocks into disjoint partition ranges of ONE PSUM tile:
            #   rows 0:64   L6(block i)    cols 0:64   of the PE array
            #   rows 64:96  L7(block i-1)  cols 64:96  (tile_position (0,64))
            #   rows 96:112 L8(block i-2)  cols 96:128 (tile_position (64,96))
            # They run concurrently (disjoint subarrays), and ONE drain + ONE
            # mask serves all three (the host staggers the m678 mask planes the
            # same way). L9(block i-2) then reads rows 96:112 after the mask.
            slots = {}
            hm5s = {}

            def slot_wins(fine):
                return (range(NSUB), MSUB) if fine else (range(NDR), DSUB)

            def emit_slot_trio(i, fine=False):
                sl = slots.setdefault(i, {})
                sl["hr678"] = hrp.tile([128, 1, BLK], f16, tag="hr678",
                                       name=f"hr678_{i}", bufs=2)
                prev = slots.get(i - 1)
                wins, wsz = slot_wins(fine)
                nmm = wsz // MSUB
                zhs = {}
                for u in wins:
                    zhs[u] = psp.tile([128, wsz], f32, tag="ps", name=f"zh_{i}_{u}")
                sl["zhs"] = zhs
                sl["fine"] = fine
                # which small layers live in slot i: slots 0..2 accumulate the
                # stagger; slots 3 (L7+L8) and 4 (L8) run inside block 3; block
                # 3's own ladder runs v3-style in the tail.
                PARTS = {0: (6,), 1: (6, 7), 2: (6, 7, 8), 3: (7, 8), 4: (8,)}
                parts = []
                if 6 in PARTS[i]:
                    parts.append((0, 64, wall[:, WOFF[6]:WOFF[6] + 64], None,
                                  lambda cs: hm5s[i][:, 0, cs]))
                if 7 in PARTS[i]:
                    parts.append((64, 96, w789[0:64, 0:32], (0, 64),
                                  lambda cs: prev["hr678"][0:64, 0, cs]))
                if 8 in PARTS[i]:
                    parts.append((96, 112, w789[64:96, 32:48], (64, 96),
                                  lambda cs: prev["hr678"][64:96, 0, cs]))
                sl["p_lo"] = parts[0][0]
                sl["p_hi"] = parts[-1][1]
                # weight-major: one LDW per weight per slot (consecutive dedup),
                # streams of different col-groups overlap on the PE
                for p0, p1, wap, tpos, rhs_of in parts:
                    for u in wins:
                        for t in range(nmm):
                            cs = slice(u * wsz + t * MSUB, u * wsz + (t + 1) * MSUB)
                            out_sl = zhs[u][p0:p1, bass.ts(t, MSUB)]
                            if tpos is None:
                                nc.tensor.matmul(out_sl, wap, rhs_of(cs),
                                                 start=True, stop=True)
                            else:
                                nc.tensor.matmul(out_sl, wap, rhs_of(cs),
                                                 start=True, stop=True,
                                                 tile_position=tpos)
                if i in hm5s:
                    del hm5s[i]

            def emit_slot_drainmask(i):
                sl = slots[i]
                p_lo, p_hi = sl["p_lo"], sl["p_hi"]
                hr678 = sl["hr678"]
                m678 = m678s[i]
                wins, wsz = slot_wins(sl["fine"])
                for u in wins:
                    dst = hr678[p_lo:p_hi, 0, u * wsz:(u + 1) * wsz]
                    drain_relu("dve" if u % 2 == 0 else "act", dst,
                               sl["zhs"][u][p_lo:p_hi, :], b678[p_lo:p_hi, 0:1])
                    # hs1 half -> gps only at coarse granularity (slack: consumers
                    # are a block away except L9, which is ~4us later)
                    mask_mul("gps" if (not sl["fine"] and u == 1) else "dve",
                             dst, dst, m678[p_lo:p_hi, 0, u * wsz:(u + 1) * wsz])
                del sl["zhs"]

            def emit_slot_l9(i):
                b = i - 2
                sl = slots[i]
                hm678 = sl["hr678"]
                fine = sl["fine"]
                osb = outp.tile([10, BLK], f32, tag="osb", bufs=2, name=f"osb_{b}")
                wins, wsz = slot_wins(fine)
                nmm = wsz // MSUB
                for u in wins:
                    z9 = psp.tile([128, wsz], f32, tag="ps", name=f"z9_{b}_{u}")
                    for t in range(nmm):
                        rhs_sl = slice(u * wsz + t * MSUB, u * wsz + (t + 1) * MSUB)
                        nc.tensor.matmul(z9[0:10, bass.ts(t, MSUB)],
                                         w789[96:112, 48:58],
                                         hm678[96:112, 0, rhs_sl],
                                         start=True, stop=True, tile_position=(96, 0))
                    if u % 2 == 0:
                        nc.scalar.activation(osb[:, u * wsz:(u + 1) * wsz],
                                             z9[0:10, :], IDENT, bias=b9[:, 0:1])
                    else:
                        nc.vector.tensor_scalar(osb[:, u * wsz:(u + 1) * wsz],
                                                z9[0:10, :], b9[:, 0:1], None, ADD)
                nc.sync.dma_start(out_d[:, bass.ts(b, BLK)], osb[:])
                if (i - 1) in slots:
                    del slots[i - 1]

            # --- schedule ---------------------------------------------------------
            # PE warmup: dummy matmuls on a memset scratch tile so the HAM clock
            # gate opens before real work arrives (weights/x still in DMA).
            nc.vector.memset(scratch[:], 0.0)
            zw = psp.tile([128, MSUB], f32, tag="ps", name="zwarm")
            for i in range(10):
                nc.tensor.matmul(zw[:], scratch[:, 0:128], scratch[:], start=True,
                                 stop=True)

            def warm(n):
                # filler matmuls that keep the PE HAM activity window busy while
                # a dependency chain stalls the real stream (tail)
                zf = psp.tile([128, MSUB], f32, tag="ps", name="zf")
                for _ in range(n):
                    nc.tensor.matmul(zf[:], scratch[:, 0:128], scratch[:],
                                     start=True, stop=True)

            issue_pack_dmas(0)
            emit_layer(0, 0)                       # L1(0) during startup
            for b in range(NBLK):
                if b + 1 < NBLK:
                    issue_pack_dmas(b + 1)
                if b == 2:
                    issue_m678_dma(4)              # phantom tail-slot planes
                if b == 3:
                    issue_m678_dma(5)
                if b >= 1:
                    warm(3)                        # bridge the block handover
                emit_layer(b, 1)                   # L2
                if b >= 1:
                    emit_slot_trio(b - 1)          # L6(b-1)+L7(b-2)+L8(b-3)
                emit_layer(b, 2)                   # L3
                if b >= 1:
                    emit_slot_drainmask(b - 1)
                emit_layer(b, 3)                   # L4
                if b == NBLK - 1:
                    emit_slot_trio(3)              # L7(2)+L8(1): no block-3 deps
                    emit_slot_drainmask(3)
                if b + 1 < NBLK:
                    emit_layer(b + 1, 0)           # L1(b+1) pipelined ahead
                if b >= 1 and b - 1 >= 2:
                    emit_slot_l9(b - 1)            # out block 0
                emit_layer(b, 4)                   # L5
                hm5s[b] = hrs.pop((b, 4))
                if b == NBLK - 1:
                    emit_slot_l9(3)                # out block 1
                    emit_slot_trio(4)              # L8(2)
                    emit_slot_drainmask(4)

            # tail: only block 3's own ladder remains (v3-style, fine windows,
            # per-window pipelining) + the last two outputs
            emit_slot_l9(4)                        # out block 2
            lad = hrp.tile([128, 1, BLK], f16, tag="hr678", name="lad3", bufs=2)
            m6d = m678s[5]                         # diagonal plane {m6,m7,m8}(3)
            LADW = [(0, 64, wall[:, WOFF[6]:WOFF[6] + 64], None, None),
                    (64, 96, w789[0:64, 0:32], (0, 64), (0, 64)),
                    (96, 112, w789[64:96, 32:48], (64, 96), (64, 96))]
            for step, (p0, p1, wap, tpos, brg) in enumerate(LADW):
                warm(4)
                for w in range(NSUB):
                    zh = psp.tile([128, MSUB], f32, tag="ps", name=f"lz_{step}_{w}")
                    cs = slice(w * MSUB, (w + 1) * MSUB)
                    rhs = (hm5s[3][:, 0, cs] if step == 0 else
                           lad[brg[0]:brg[1], 0, cs])
                    if tpos is None:
                        nc.tensor.matmul(zh[p0:p1, :], wap, rhs, start=True,
                                         stop=True)
                    else:
                        nc.tensor.matmul(zh[p0:p1, :], wap, rhs, start=True,
                                         stop=True, tile_position=tpos)
                    dst = lad[p0:p1, 0, cs]
                    drain_relu("dve" if w % 2 == 0 else "act", dst, zh[p0:p1, :],
                               b678[p0:p1, 0:1])
                    mask_mul("dve", dst, dst, m6d[p0:p1, 0, cs])
            warm(4)
            slots[5] = {"hr678": lad, "fine": True, "p_lo": 0, "p_hi": 112}
            emit_slot_l9(5)                        # out block 3

    _dedup_ldweights(nc)
    nc.compile()
    return nc


def _get_program():
    if "nc" not in _PROG:
        _PROG["nc"] = _build_program()
    return _PROG["nc"]


def _host_prep(inputs):
    """Build per-core input maps (numpy only)."""
    x = np.asarray(inputs["x"], dtype=np.float32)
    Ws = [np.asarray(inputs[f"W{i}"], dtype=np.float32) for i in range(1, 10)]
    bs = [np.asarray(inputs[f"b{i}"], dtype=np.float32) for i in range(1, 10)]
    ms = [np.asarray(inputs[f"m{i}"], dtype=np.float32) for i in range(1, 9)]

    # fold dropout scale into next layer's weights; binarize masks
    Wf = [Ws[0]]
    for i in range(1, 9):
        s = float(ms[i - 1].max())
        if s <= 0.0:  # degenerate all-dropped mask; keep weights unscaled
            s = 1.0
        Wf.append(Ws[i] * np.float32(s))

    # weight blob: w1@0 w2@256 w3@512 w4@1536 w5@2560 w6@2816 w789@2880
    WOFF = {1: 0, 2: 256, 3: 512, 4: 1536, 5: 2560, 6: 2816, 789: 2880}
    wb = np.zeros((128, 2944), dtype=np.float16)
    for l in range(1, 7):
        W = Wf[l - 1]
        K, N = W.shape
        for k in range((K + 127) // 128):
            blk = W[k * 128:(k + 1) * 128].astype(np.float16)
            wb[: blk.shape[0], WOFF[l] + k * N: WOFF[l] + k * N + N] = blk
    wb[0:64, 2880:2912] = Wf[6].astype(np.float16)    # W7
    wb[64:96, 2912:2928] = Wf[7].astype(np.float16)   # W8
    wb[96:112, 2928:2938] = Wf[8].astype(np.float16)  # W9
    wb1, wb = np.ascontiguousarray(wb[:, 0:256]), np.ascontiguousarray(wb[:, 256:])
    bb = np.zeros((128, 12), dtype=np.float32)
    bb[:, 0] = bs[0]
    bb[:, 1], bb[:, 2] = bs[1][0:128], bs[1][128:256]
    for c in range(4):
        bb[:, 3 + c] = bs[2][c * 128:(c + 1) * 128]
    bb[:, 7], bb[:, 8] = bs[3][0:128], bs[3][128:256]
    bb[:, 9] = bs[4]
    bb[0:64, 10], bb[64:96, 10], bb[96:112, 10] = bs[5], bs[6], bs[7]
    bb[0:10, 11] = bs[8]
    shared = {"WB1": wb1, "WB": wb, "BB": bb}

    in_maps = []
    for c in range(NCORES):
        sl = slice(c * SHARD, (c + 1) * SHARD)
        pack = np.zeros((NBLK, 128, NPACK, BLK), dtype=np.float16)
        m678 = np.zeros((NSLOT, 128, 1, BLK), dtype=np.float16)
        xT = x[sl].T  # (256, SHARD)
        mT = [None] + [(ms[i][sl] != 0).T.astype(np.float16) for i in range(8)]
        for b in range(NBLK):
            cs = slice(b * BLK, (b + 1) * BLK)
            pack[b, :, 0, :] = xT[0:128, cs]
            pack[b, :, 1, :] = xT[128:256, cs]
            pack[b, :, 2, :] = mT[1][:, cs]
            pack[b, :, 3, :], pack[b, :, 4, :] = mT[2][0:128, cs], mT[2][128:256, cs]
            for k in range(4):
                pack[b, :, 5 + k, :] = mT[3][k * 128:(k + 1) * 128, cs]
            pack[b, :, 9, :], pack[b, :, 10, :] = mT[4][0:128, cs], mT[4][128:256, cs]
            pack[b, :, 11, :] = mT[5][:, cs]
        # slot-staggered m678 planes: slot i masks {m6(i), m7(i-1), m8(i-2)};
        # plane 5 is block 3's diagonal {m6(3), m7(3), m8(3)} for the tail ladder
        for i in range(NSLOT - 1):
            if i < NBLK:
                m678[i, 0:64, 0, :] = mT[6][:, i * BLK:(i + 1) * BLK]
            if 0 <= i - 1 < NBLK:
                m678[i, 64:96, 0, :] = mT[7][:, (i - 1) * BLK:i * BLK]
            if 0 <= i - 2 < NBLK:
                m678[i, 96:112, 0, :] = mT[8][:, (i - 2) * BLK:(i - 1) * BLK]
        lb = slice((NBLK - 1) * BLK, NBLK * BLK)
        m678[5, 0:64, 0, :] = mT[6][:, lb]
        m678[5, 64:96, 0, :] = mT[7][:, lb]
        m678[5, 96:112, 0, :] = mT[8][:, lb]
        in_maps.append({"pack": pack, "M678": m678, **shared})
    return in_maps


def kernel(**inputs) -> np.ndarray:
    from concourse.bass_utils import run_bass_kernel_spmd

    nc = _get_program()
    in_maps = _host_prep(inputs)
    res = run_bass_kernel_spmd(nc, in_maps, list(range(NCORES)))
    out = np.empty((BATCH, DIMS[-1]), dtype=np.float32)
    for c in range(NCORES):
        out[c * SHARD:(c + 1) * SHARD, :] = res.results[c]["outT"].T
    return out


# revision 22
# speedup vs baseline: 1.1980x; 1.0382x over previous
"""Trainium2 Bass kernel for the 9-layer dense MLP (dropout-mask training forward).

Strategy (pure data parallel, 8 cores, 8192 batch rows each):
  - Activations kept transposed on-chip: features on partitions, batch cols on free dim.
    Each layer computes zT = W^T @ hT via nc.tensor.matmul(out, lhsT=W, rhs=hT).
  - fp16 weights/activations/masks (fp32 PSUM accumulation), fp32 biases + output.
  - Dropout masks binarized on host ({0,1} fp16); the 1/keep scale is folded into the
    next layer's weights.
  - Host pack layout [NBLK, 128, NPACK, BLK] so each per-block DMA is contiguous per
    partition (2 DMAs per block: x+m1, then the remaining masks).
  - PSUM: one shared pool of 4x [128,1024] fp32 tiles (8 banks). Matmuls write 512-col
    halves; drains are single FD=1024 instructions (fused bias+relu) split ~3:1
    ACT:DVE; mask multiplies are FD=2048 DVE tensor_tensor with a measured dose
    offloaded to GpSimd.
  - Small layers 6/7/8 partition-packed (offsets 0/64/96 via matmul tile_position);
    each ladder step drains immediately to SBUF so its PSUM tile recycles fast.
    Block b's ladder is software-pipelined into block b+1's big-layer bursts.
  - A short burst of dummy matmuls at t~1us keeps the PE HAM activity monitor busy so
    the array is at full clock (K=8/8) when real work arrives.
"""

import sys

sys.path.insert(0, "/opt/trn_rl_repo")

import numpy as np

DIMS = [256, 128, 256, 512, 256, 128, 64, 32, 16, 10]
NCORES = 8
BATCH = 65536
SHARD = BATCH // NCORES  # 8192
MSUB = 512               # matmul N (PSUM bank limit for fp32)
DSUB = 1024              # drain granularity (2 banks)
BLK = 2048               # block columns
NBLK = SHARD // BLK      # 4
NSUB = BLK // MSUB       # 4
NDR = BLK // DSUB        # 2

# pack chunk layout (each chunk = 128 partitions x BLK cols, fp16), per block:
#   0,1: xT   2: m1   3,4: m2   5-8: m3   9,10: m4   11: m5
# m678 ships separately as NBLK+2 slot-staggered planes (rows 0:64 = m6(slot),
# 64:96 = m7(slot-1), 96:112 = m8(slot-2)) to match the slot-fused ladder.
NPACK = 12
NSLOT = NBLK + 2

_PROG = {}


def _raise_sbuf_cap():
    # tile_utils.max_sbuf_usage is a stale 192KB constant; cayman has 208KB usable.
    import concourse.tile_utils as tu

    if getattr(tu, "max_sbuf_usage", 0) < 206 * 1024:
        tu.max_sbuf_usage = 206 * 1024


def _dedup_ldweights(nc):
    """Remove back-to-back redundant LDWEIGHTS (same stationary operand) so
    consecutive same-weight matmuls pipeline on the PE. Only drops LDW
    instructions that carry no semaphore waits/updates."""
    removed = 0
    for fn in nc.m.functions:
        for blk in fn.blocks:
            il = blk.instructions
            keep, last_sig = [], None
            for inst in il:
                nm = type(inst).__name__
                if nm == "InstLdweights":
                    sig = (str(inst.ins[0]), str(inst.is_transpose), str(inst.perf_mode),
                           str(getattr(inst, "tile_position", None)))
                    si = inst.sync_info
                    clean = si is None or (not si.on_wait and not si.on_update)
                    if sig == last_sig and clean:
                        removed += 1
                        continue
                    last_sig = sig
                keep.append(inst)
            if removed and len(keep) != len(il):
                while il:
                    il.pop()
                il.extend(keep)
    return removed


def _build_program():
    import concourse.bass as bass
    import concourse.tile as tile
    from concourse import bacc, mybir

    _raise_sbuf_cap()

    f16 = mybir.dt.float16
    f32 = mybir.dt.float32
    RELU = mybir.ActivationFunctionType.Relu
    IDENT = mybir.ActivationFunctionType.Identity
    ADD = mybir.AluOpType.add
    MAX = mybir.AluOpType.max

    nc = bacc.Bacc("TRN2", target_bir_lowering=False, debug=False, num_devices=NCORES)

    pack_d = nc.dram_tensor("pack", [NBLK, 128, NPACK, BLK], f16, kind="ExternalInput").ap()
    m678_d = nc.dram_tensor("M678", [NSLOT, 128, 1, BLK], f16, kind="ExternalInput").ap()
    # weights in two host-laid-out fp16 blobs (W1 separate so the first
    # LDWEIGHTS isn't gated on the full blob), biases in one fp32 blob
    wb1_d = nc.dram_tensor("WB1", [128, 256], f16, kind="ExternalInput").ap()
    wb_d = nc.dram_tensor("WB", [128, 2688], f16, kind="ExternalInput").ap()
    bb_d = nc.dram_tensor("BB", [128, 12], f32, kind="ExternalInput").ap()
    out_d = nc.dram_tensor("outT", [10, SHARD], f32, kind="ExternalOutput").ap()

    with tile.TileContext(nc) as tc:
        with (
            tc.tile_pool(name="wpool", bufs=1) as wp,
            tc.tile_pool(name="mk", bufs=2) as mkp,
            tc.tile_pool(name="hr", bufs=1) as hrp,
            tc.tile_pool(name="osb", bufs=2) as outp,
            tc.tile_pool(name="ps", bufs=4, space="PSUM") as psp,
        ):
            wall = wp.tile([128, 2944], f16, tag="wall")
            ball = wp.tile([128, 12], f32, tag="ball")
            scratch = wp.tile([128, 512], f16, tag="scratch")
            # blob column offsets: w1@0(256) w2@256(256) w3@512(1024) w4@1536(1024)
            #   w5@2560(256) w6@2816(64) w789@2880(64: W7 r0-63 c0-31, W8 r64-95
            #   c32-47, W9 r96-111 c48-57)
            WOFF = {1: 0, 2: 256, 3: 512, 4: 1536, 5: 2560, 6: 2816, 789: 2880}
            w789 = wall[:, WOFF[789]:WOFF[789] + 64]
            b15 = ball[:, 0:10]
            b678 = ball[:, 10:11]
            b9 = ball[0:10, 11:12]

            def wslice(l, k, c, N):
                base = WOFF[l] + k * N
                return wall[:, base + c * 128: base + (c + 1) * 128]

            def drain_relu(eng, dst, zsrc, bias_ap):
                if eng == "act":
                    nc.scalar.activation(dst, zsrc, RELU, bias=bias_ap)
                else:
                    nc.vector.tensor_scalar(dst, zsrc, bias_ap, 0.0, ADD, MAX)

            def mask_mul(eng, dst, src, msrc):
                if eng == "gps":
                    nc.gpsimd.tensor_mul(dst, src, msrc)
                else:
                    nc.vector.tensor_mul(dst, src, msrc)

            # drain engine picker: ~70:30 act:dve (ACT is cheaper per element but
            # DVE has mask work too; this balances their queues)
            dr_i = [0]

            def pick_drain():
                i = dr_i[0]
                dr_i[0] += 1
                return "dve" if i % 10 in (2, 5, 8) else "act"

            state = {}
            packs = {}
            hrs = {}

            def issue_pack_dmas(b):
                # per-chunk tiles/DMAs: each mask tile's ring slot is released as
                # soon as its own layer consumes it, so block b+2's DMAs start
                # early instead of waiting for ALL of block b's masks (WAR).
                pk3 = mkp.tile([128, 3, BLK], f16, tag="pk3", name=f"pk3_{b}")
                m2t = mkp.tile([128, 2, BLK], f16, tag="m2", name=f"m2_{b}")
                m3t = mkp.tile([128, 4, BLK], f16, tag="m3", name=f"m3_{b}")
                m4t = mkp.tile([128, 2, BLK], f16, tag="m4", name=f"m4_{b}")
                m5t = mkp.tile([128, 1, BLK], f16, tag="m5", name=f"m5_{b}")
                if b == 0:
                    nc.sync.dma_start(wall[:, 0:256], wb1_d[:])
                    nc.sync.dma_start(ball[:], bb_d[:])
                    nc.sync.dma_start(pk3[:, :, 0:DSUB], pack_d[0, :, 0:3, 0:DSUB])
                    nc.sync.dma_start(pk3[:, :, DSUB:BLK], pack_d[0, :, 0:3, DSUB:BLK])
                    nc.sync.dma_start(wall[:, 256:], wb_d[:])
                else:
                    nc.sync.dma_start(pk3[:], pack_d[b, :, 0:3, :])
                nc.sync.dma_start(m2t[:], pack_d[b, :, 3:5, :])
                nc.sync.dma_start(m3t[:], pack_d[b, :, 5:9, :])
                nc.sync.dma_start(m4t[:], pack_d[b, :, 9:11, :])
                nc.sync.dma_start(m5t[:], pack_d[b, :, 11:12, :])
                packs[b] = (pk3, m2t, m3t, m4t, m5t)
                issue_m678_dma(b)

            m678s = {}

            def issue_m678_dma(i):
                m678s[i] = mkp.tile([128, 1, BLK], f16, tag="m678", name=f"m678s_{i}",
                                    bufs=3)
                nc.sync.dma_start(m678s[i][:], m678_d[i])

            # (Kc, layer, wN, Cc, bias_off, hrtag)
            LAYER_CFG = [
                (2, 1, 128, 1, 0, "hr1"),
                (1, 2, 256, 2, 1, "hr2"),
                (2, 3, 512, 4, 3, "hr3"),
                (4, 4, 256, 2, 7, "hr4"),
                (2, 5, 128, 1, 9, "hr5"),
            ]
            # mask engine schedule per layer index. GpSimd is ~4x slower than DVE
            # per element, so it only gets masks with slack before their consumer:
            # m1 (L1 runs a block ahead) and m5 (ladder consumes it next block).
            # L2/L3/L4 masks sit on the next layer's critical path -> DVE only.
            MASK_MODE = {
                0: ["dve2"],
                1: ["dve2", "dve2"],
                2: ["dve2", "dve2", "dve2", "dve2"],
                3: ["dve2", "dve2"],
                4: ["dve2"],
            }
            # per-layer drain engine assignment, in (u, c) order. ACT-heavy for
            # the mid-block bulk (L3/L4); the boundary-critical L2 keeps one on
            # DVE so its chain isn't stuck behind the ACT queue.
            DRAIN_ENG = {
                0: ["dve", "act"],
                1: ["act", "act", "dve", "act"],
                2: ["act", "act", "act", "dve", "act", "act", "act", "act"],
                3: ["act", "act", "act", "act"],
                4: ["dve", "act"],
            }

            def emit_layer(b, li):
                Kc, wl, wN, Cc, boff, hrtag = LAYER_CFG[li]
                hr = hrp.tile([128, Cc, BLK], f16, tag=hrtag, name=hrtag + f"_{b}",
                              bufs=2 if hrtag in ("hr5", "hr2", "hr1") else 1)
                pk3 = packs[b][0]
                hin = pk3 if li == 0 else hrs[(b, li - 1)]

                def msl(c, cols):
                    if li == 0:
                        return pk3[:, 2, cols]
                    return packs[b][li][:, c, cols]

                zs = {}
                for c in range(Cc):
                    for u in range(NDR):
                        zs[c, u] = psp.tile([128, DSUB], f32, tag="ps",
                                            name=f"z_{hrtag}_{b}_{c}_{u}")
                if b == 0 and li == 0:
                    # startup: u-outer so each 1024-col window flows MM -> drain
                    # -> mask as early as possible
                    for u in range(NDR):
                        for k in range(Kc):
                            for t in range(2):
                                nc.tensor.matmul(
                                    zs[0, u][:, bass.ts(t, MSUB)],
                                    wslice(wl, k, 0, wN),
                                    hin[:, k, u * DSUB + t * MSUB:
                                        u * DSUB + (t + 1) * MSUB],
                                    start=(k == 0), stop=(k == Kc - 1))
                        drain_relu("dve" if u == 0 else "act",
                                   hr[:, 0, bass.ts(u, DSUB)], zs[0, u][:],
                                   b15[:, 0:1])
                        mask_mul("dve", hr[:, 0, bass.ts(u, DSUB)],
                                 hr[:, 0, bass.ts(u, DSUB)],
                                 msl(0, bass.ts(u, DSUB)))
                    hrs[(b, li)] = hr
                    return
                # weight-major matmuls so consecutive MMs share one LDWEIGHTS
                for c in range(Cc):
                    for k in range(Kc):
                        wap = wslice(wl, k, c, wN)
                        for t in range(NSUB):
                            nc.tensor.matmul(
                                zs[c, t // 2][:, bass.ts(t % 2, MSUB)], wap,
                                hin[:, k, bass.ts(t, MSUB)],
                                start=(k == 0), stop=(k == Kc - 1))
                de = DRAIN_ENG[li]
                for u in range(NDR):
                    for c in range(Cc):
                        drain_relu(de[u * Cc + c], hr[:, c, bass.ts(u, DSUB)],
                                   zs[c, u][:], b15[:, boff + c:boff + c + 1])
                full = slice(0, BLK)
                for c in range(Cc):
                    mode = MASK_MODE[li][c]
                    if mode == "dve":
                        mask_mul("dve", hr[:, c, full], hr[:, c, full], msl(c, full))
                    elif mode == "dve2":
                        for uu in range(NDR):
                            hs = bass.ts(uu, DSUB)
                            mask_mul("dve", hr[:, c, hs], hr[:, c, hs], msl(c, hs))
                    else:
                        hs0, hs1 = bass.ts(0, DSUB), bass.ts(1, DSUB)
                        mask_mul("dve", hr[:, c, hs0], hr[:, c, hs0], msl(c, hs0))
                        mask_mul("gps", hr[:, c, hs1], hr[:, c, hs1], msl(c, hs1))
                hrs[(b, li)] = hr
                if li > 0:
                    del hrs[(b, li - 1)]

            # --- small-layer ladder, slot-fused across blocks --------------------
            # Slot i co-issues three INDEPENDENT small matmuls from staggered
            # blocks into disjoint partition ranges of ONE PSUM tile:
            #   rows 0:64   L6(block i)    cols 0:64   of the PE array
            #   rows 64:96  L7(block i-1)  cols 64:96  (tile_position (0,64))
            #   rows 96:112 L8(block i-2)  cols 96:128 (tile_position (64,96))
            # They run concurrently (disjoint subarrays), and ONE drain + ONE
            # mask serves all three (the host staggers the m678 mask planes the
            # same way). L9(block i-2) then reads rows 96:112 after the mask.
            slots = {}
            hm5s = {}

            def slot_wins(fine):
                return (range(NSUB), MSUB) if fine else (range(NDR), DSUB)

            def emit_slot_trio(i, fine=False):
                sl = slots.setdefault(i, {})
                sl["hr678"] = hrp.tile([128, 1, BLK], f16, tag="hr678",
                                       name=f"hr678_{i}", bufs=2)
                prev = slots.get(i - 1)
                wins, wsz = slot_wins(fine)
                nmm = wsz // MSUB
                zhs = {}
                for u in wins:
                    zhs[u] = psp.tile([128, wsz], f32, tag="ps", name=f"zh_{i}_{u}")
                sl["zhs"] = zhs
                sl["fine"] = fine
                # which small layers live in slot i: slots 0..2 accumulate the
                # stagger; slots 3 (L7+L8) and 4 (L8) run inside block 3; block
                # 3's own ladder runs v3-style in the tail.
                PARTS = {0: (6,), 1: (6, 7), 2: (6, 7, 8), 3: (7, 8), 4: (8,)}
                parts = []
                if 6 in PARTS[i]:
                    parts.append((0, 64, wall[:, WOFF[6]:WOFF[6] + 64], None,
                                  lambda cs: hm5s[i][:, 0, cs]))
                if 7 in PARTS[i]:
                    parts.append((64, 96, w789[0:64, 0:32], (0, 64),
                                  lambda cs: prev["hr678"][0:64, 0, cs]))
                if 8 in PARTS[i]:
                    parts.append((96, 112, w789[64:96, 32:48], (64, 96),
                                  lambda cs: prev["hr678"][64:96, 0, cs]))
                sl["p_lo"] = parts[0][0]
                sl["p_hi"] = parts[-1][1]
                # weight-major: one LDW per weight per slot (consecutive dedup),
                # streams of different col-groups overlap on the PE
                for p0, p1, wap, tpos, rhs_of in parts:
                    for u in wins:
                        for t in range(nmm):
                            cs = slice(u * wsz + t * MSUB, u * wsz + (t + 1) * MSUB)
                            out_sl = zhs[u][p0:p1, bass.ts(t, MSUB)]
                            if tpos is None:
                                nc.tensor.matmul(out_sl, wap, rhs_of(cs),
                                                 start=True, stop=True)
                            else:
                                nc.tensor.matmul(out_sl, wap, rhs_of(cs),
                                                 start=True, stop=True,
                                                 tile_position=tpos)
                if i in hm5s:
                    del hm5s[i]

            def emit_slot_drainmask(i):
                sl = slots[i]
                p_lo, p_hi = sl["p_lo"], sl["p_hi"]
                hr678 = sl["hr678"]
                m678 = m678s[i]
                wins, wsz = slot_wins(sl["fine"])
                for u in wins:
                    dst = hr678[p_lo:p_hi, 0, u * wsz:(u + 1) * wsz]
                    drain_relu("dve" if u % 2 == 0 else "act", dst,
                               sl["zhs"][u][p_lo:p_hi, :], b678[p_lo:p_hi, 0:1])
                    # hs1 half -> gps only at coarse granularity (slack: consumers
                    # are a block away except L9, which is ~4us later)
                    mask_mul("gps" if (not sl["fine"] and u == 1) else "dve",
                             dst, dst, m678[p_lo:p_hi, 0, u * wsz:(u + 1) * wsz])
                del sl["zhs"]

            def emit_slot_l9(i):
                b = i - 2
                sl = slots[i]
                hm678 = sl["hr678"]
                fine = sl["fine"]
                osb = outp.tile([10, BLK], f32, tag="osb", bufs=2, name=f"osb_{b}")
                wins, wsz = slot_wins(fine)
                nmm = wsz // MSUB
                for u in wins:
                    z9 = psp.tile([128, wsz], f32, tag="ps", name=f"z9_{b}_{u}")
                    for t in range(nmm):
                        rhs_sl = slice(u * wsz + t * MSUB, u * wsz + (t + 1) * MSUB)
                        nc.tensor.matmul(z9[0:10, bass.ts(t, MSUB)],
                                         w789[96:112, 48:58],
                                         hm678[96:112, 0, rhs_sl],
                                         start=True, stop=True, tile_position=(96, 0))
                    if u % 2 == 0:
                        nc.scalar.activation(osb[:, u * wsz:(u + 1) * wsz],
                                             z9[0:10, :], IDENT, bias=b9[:, 0:1])
                    else:
                        nc.vector.tensor_scalar(osb[:, u * wsz:(u + 1) * wsz],
                                                z9[0:10, :], b9[:, 0:1], None, ADD)
                nc.sync.dma_start(out_d[:, bass.ts(b, BLK)], osb[:])
                if (i - 1) in slots:
                    del slots[i - 1]

            # --- schedule ---------------------------------------------------------
            # PE warmup: dummy matmuls on a memset scratch tile so the HAM clock
            # gate opens before real work arrives (weights/x still in DMA).
            nc.vector.memset(scratch[:], 0.0)
            zw = psp.tile([128, MSUB], f32, tag="ps", name="zwarm")
            for i in range(10):
                nc.tensor.matmul(zw[:], scratch[:, 0:128], scratch[:], start=True,
                                 stop=True)

            def warm(n):
                # filler matmuls that keep the PE HAM activity window busy while
                # a dependency chain stalls the real stream (tail)
                zf = psp.tile([128, MSUB], f32, tag="ps", name="zf")
                for _ in range(n):
                    nc.tensor.matmul(zf[:], scratch[:, 0:128], scratch[:],
                                     start=True, stop=True)

            issue_pack_dmas(0)
            emit_layer(0, 0)                       # L1(0) during startup
            for b in range(NBLK):
                if b + 1 < NBLK:
                    issue_pack_dmas(b + 1)
                if b == 2:
                    issue_m678_dma(4)              # phantom tail-slot planes
                if b == 3:
                    issue_m678_dma(5)
                if b >= 1:
                    warm(3)                        # bridge the block handover
                emit_layer(b, 1)                   # L2
                if b >= 1:
                    emit_slot_trio(b - 1)          # L6(b-1)+L7(b-2)+L8(b-3)
                emit_layer(b, 2)                   # L3
                if b >= 1:
                    emit_slot_drainmask(b - 1)
                emit_layer(b, 3)                   # L4
                if b == NBLK - 1:
                    emit_slot_trio(3)              # L7(2)+L8(1): no block-3 deps
                    emit_slot_drainmask(3)
                if b + 1 < NBLK:
                    emit_layer(b + 1, 0)           # L1(b+1) pipelined ahead
                if b >= 1 and b - 1 >= 2:
                    emit_slot_l9(b - 1)            # out block 0
                emit_layer(b, 4)                   # L5
                hm5s[b] = hrs.pop((b, 4))
                if b == NBLK - 1:
                    emit_slot_l9(3)                # out block 1
                    warm(4)
                    emit_slot_trio(4)              # L8(2)
                    emit_slot_drainmask(4)

            # tail: only block 3's own ladder remains (v3-style, fine windows,
            # per-window pipelining) + the last two outputs
            warm(4)
            emit_slot_l9(4)                        # out block 2
            lad = hrp.tile([128, 1, BLK], f16, tag="hr678", name="lad3", bufs=2)
            m6d = m678s[5]                         # diagonal plane {m6,m7,m8}(3)
            LADW = [(0, 64, wall[:, WOFF[6]:WOFF[6] + 64], None, None),
                    (64, 96, w789[0:64, 0:32], (0, 64), (0, 64)),
                    (96, 112, w789[64:96, 32:48], (64, 96), (64, 96))]
            for step, (p0, p1, wap, tpos, brg) in enumerate(LADW):
                warm(4)
                for w in range(NSUB):
                    zh = psp.tile([128, MSUB], f32, tag="ps", name=f"lz_{step}_{w}")
                    cs = slice(w * MSUB, (w + 1) * MSUB)
                    rhs = (hm5s[3][:, 0, cs] if step == 0 else
                           lad[brg[0]:brg[1], 0, cs])
                    if tpos is None:
                        nc.tensor.matmul(zh[p0:p1, :], wap, rhs, start=True,
                                         stop=True)
                    else:
                        nc.tensor.matmul(zh[p0:p1, :], wap, rhs, start=True,
                                         stop=True, tile_position=tpos)
                    dst = lad[p0:p1, 0, cs]
                    drain_relu("dve" if w % 2 == 0 else "act", dst, zh[p0:p1, :],
                               b678[p0:p1, 0:1])
                    mask_mul("dve", dst, dst, m6d[p0:p1, 0, cs])
            warm(4)
            slots[5] = {"hr678": lad, "fine": True, "p_lo": 0, "p_hi": 112}
            emit_slot_l9(5)                        # out block 3

    _dedup_ldweights(nc)
    nc.compile()
    return nc


def _get_program():
    if "nc" not in _PROG:
        _PROG["nc"] = _build_program()
    return _PROG["nc"]


def _host_prep(inputs):
    """Build per-core input maps (numpy only)."""
    x = np.asarray(inputs["x"], dtype=np.float32)
    Ws = [np.asarray(inputs[f"W{i}"], dtype=np.float32) for i in range(1, 10)]
    bs = [np.asarray(inputs[f"b{i}"], dtype=np.float32) for i in range(1, 10)]
    ms = [np.asarray(inputs[f"m{i}"], dtype=np.float32) for i in range(1, 9)]

    # fold dropout scale into next layer's weights; binarize masks
    Wf = [Ws[0]]
    for i in range(1, 9):
        s = float(ms[i - 1].max())
        if s <= 0.0:  # degenerate all-dropped mask; keep weights unscaled
            s = 1.0
        Wf.append(Ws[i] * np.float32(s))

    # weight blob: w1@0 w2@256 w3@512 w4@1536 w5@2560 w6@2816 w789@2880
    WOFF = {1: 0, 2: 256, 3: 512, 4: 1536, 5: 2560, 6: 2816, 789: 2880}
    wb = np.zeros((128, 2944), dtype=np.float16)
    for l in range(1, 7):
        W = Wf[l - 1]
        K, N = W.shape
        for k in range((K + 127) // 128):
            blk = W[k * 128:(k + 1) * 128].astype(np.float16)
            wb[: blk.shape[0], WOFF[l] + k * N: WOFF[l] + k * N + N] = blk
    wb[0:64, 2880:2912] = Wf[6].astype(np.float16)    # W7
    wb[64:96, 2912:2928] = Wf[7].astype(np.float16)   # W8
    wb[96:112, 2928:2938] = Wf[8].astype(np.float16)  # W9
    wb1, wb = np.ascontiguousarray(wb[:, 0:256]), np.ascontiguousarray(wb[:, 256:])
    bb = np.zeros((128, 12), dtype=np.float32)
    bb[:, 0] = bs[0]
    bb[:, 1], bb[:, 2] = bs[1][0:128], bs[1][128:256]
    for c in range(4):
        bb[:, 3 + c] = bs[2][c * 128:(c + 1) * 128]
    bb[:, 7], bb[:, 8] = bs[3][0:128], bs[3][128:256]
    bb[:, 9] = bs[4]
    bb[0:64, 10], bb[64:96, 10], bb[96:112, 10] = bs[5], bs[6], bs[7]
    bb[0:10, 11] = bs[8]
    shared = {"WB1": wb1, "WB": wb, "BB": bb}

    in_maps = []
    for c in range(NCORES):
        sl = slice(c * SHARD, (c + 1) * SHARD)
        pack = np.zeros((NBLK, 128, NPACK, BLK), dtype=np.float16)
        m678 = np.zeros((NSLOT, 128, 1, BLK), dtype=np.float16)
        xT = x[sl].T  # (256, SHARD)
        mT = [None] + [(ms[i][sl] != 0).T.astype(np.float16) for i in range(8)]
        for b in range(NBLK):
            cs = slice(b * BLK, (b + 1) * BLK)
            pack[b, :, 0, :] = xT[0:128, cs]
            pack[b, :, 1, :] = xT[128:256, cs]
            pack[b, :, 2, :] = mT[1][:, cs]
            pack[b, :, 3, :], pack[b, :, 4, :] = mT[2][0:128, cs], mT[2][128:256, cs]
            for k in range(4):
                pack[b, :, 5 + k, :] = mT[3][k * 128:(k + 1) * 128, cs]
            pack[b, :, 9, :], pack[b, :, 10, :] = mT[4][0:128, cs], mT[4][128:256, cs]
            pack[b, :, 11, :] = mT[5][:, cs]
        # slot-staggered m678 planes: slot i masks {m6(i), m7(i-1), m8(i-2)};
        # plane 5 is block 3's diagonal {m6(3), m7(3), m8(3)} for the tail ladder
        for i in range(NSLOT - 1):
            if i < NBLK:
                m678[i, 0:64, 0, :] = mT[6][:, i * BLK:(i + 1) * BLK]
            if 0 <= i - 1 < NBLK:
                m678[i, 64:96, 0, :] = mT[7][:, (i - 1) * BLK:i * BLK]
            if 0 <= i - 2 < NBLK:
                m678[i, 96:112, 0, :] = mT[8][:, (i - 2) * BLK:(i - 1) * BLK]
        lb = slice((NBLK - 1) * BLK, NBLK * BLK)
        m678[5, 0:64, 0, :] = mT[6][:, lb]
        m678[5, 64:96, 0, :] = mT[7][:, lb]
        m678[5, 96:112, 0, :] = mT[8][:, lb]
        in_maps.append({"pack": pack, "M678": m678, **shared})
    return in_maps


def kernel(**inputs) -> np.ndarray:
    from concourse.bass_utils import run_bass_kernel_spmd

    nc = _get_program()
    in_maps = _host_prep(inputs)
    res = run_bass_kernel_spmd(nc, in_maps, list(range(NCORES)))
    out = np.empty((BATCH, DIMS[-1]), dtype=np.float32)
    for c in range(NCORES):
        out[c * SHARD:(c + 1) * SHARD, :] = res.results[c]["outT"].T
    return out


# revision 23
# speedup vs baseline: 1.2271x; 1.0243x over previous
"""Trainium2 Bass kernel for the 9-layer dense MLP (dropout-mask training forward).

Strategy (pure data parallel, 8 cores, 8192 batch rows each):
  - Activations kept transposed on-chip: features on partitions, batch cols on free dim.
    Each layer computes zT = W^T @ hT via nc.tensor.matmul(out, lhsT=W, rhs=hT).
  - fp16 weights/activations/masks (fp32 PSUM accumulation), fp32 biases + output.
  - Dropout masks binarized on host ({0,1} fp16); the 1/keep scale is folded into the
    next layer's weights.
  - Host pack layout [NBLK, 128, NPACK, BLK] so each per-block DMA is contiguous per
    partition (2 DMAs per block: x+m1, then the remaining masks).
  - PSUM: one shared pool of 4x [128,1024] fp32 tiles (8 banks). Matmuls write 512-col
    halves; drains are single FD=1024 instructions (fused bias+relu) split ~3:1
    ACT:DVE; mask multiplies are FD=2048 DVE tensor_tensor with a measured dose
    offloaded to GpSimd.
  - Small layers 6/7/8 partition-packed (offsets 0/64/96 via matmul tile_position);
    each ladder step drains immediately to SBUF so its PSUM tile recycles fast.
    Block b's ladder is software-pipelined into block b+1's big-layer bursts.
  - A short burst of dummy matmuls at t~1us keeps the PE HAM activity monitor busy so
    the array is at full clock (K=8/8) when real work arrives.
"""

import sys

sys.path.insert(0, "/opt/trn_rl_repo")

import numpy as np

DIMS = [256, 128, 256, 512, 256, 128, 64, 32, 16, 10]
NCORES = 8
BATCH = 65536
SHARD = BATCH // NCORES  # 8192
MSUB = 512               # matmul N (PSUM bank limit for fp32)
DSUB = 1024              # drain granularity (2 banks)
BLK = 2048               # block columns
NBLK = SHARD // BLK      # 4
NSUB = BLK // MSUB       # 4
NDR = BLK // DSUB        # 2

# pack chunk layout (each chunk = 128 partitions x BLK cols, fp16), per block:
#   0,1: xT   2: m1   3,4: m2   5-8: m3   9,10: m4   11: m5
# m678 ships separately as NBLK+2 slot-staggered planes (rows 0:64 = m6(slot),
# 64:96 = m7(slot-1), 96:112 = m8(slot-2)) to match the slot-fused ladder.
NPACK = 12
NSLOT = NBLK + 2

_PROG = {}


def _raise_sbuf_cap():
    # tile_utils.max_sbuf_usage is a stale 192KB constant; cayman has 208KB usable.
    import concourse.tile_utils as tu

    if getattr(tu, "max_sbuf_usage", 0) < 206 * 1024:
        tu.max_sbuf_usage = 206 * 1024


def _dedup_ldweights(nc):
    """Remove back-to-back redundant LDWEIGHTS (same stationary operand) so
    consecutive same-weight matmuls pipeline on the PE. Only drops LDW
    instructions that carry no semaphore waits/updates."""
    removed = 0
    for fn in nc.m.functions:
        for blk in fn.blocks:
            il = blk.instructions
            keep, last_sig = [], None
            for inst in il:
                nm = type(inst).__name__
                if nm == "InstLdweights":
                    sig = (str(inst.ins[0]), str(inst.is_transpose), str(inst.perf_mode),
                           str(getattr(inst, "tile_position", None)))
                    si = inst.sync_info
                    clean = si is None or (not si.on_wait and not si.on_update)
                    if sig == last_sig and clean:
                        removed += 1
                        continue
                    last_sig = sig
                keep.append(inst)
            if removed and len(keep) != len(il):
                while il:
                    il.pop()
                il.extend(keep)
    return removed


def _build_program():
    import concourse.bass as bass
    import concourse.tile as tile
    from concourse import bacc, mybir

    _raise_sbuf_cap()

    f16 = mybir.dt.float16
    f32 = mybir.dt.float32
    RELU = mybir.ActivationFunctionType.Relu
    IDENT = mybir.ActivationFunctionType.Identity
    ADD = mybir.AluOpType.add
    MAX = mybir.AluOpType.max

    nc = bacc.Bacc("TRN2", target_bir_lowering=False, debug=False, num_devices=NCORES)

    pack_d = nc.dram_tensor("pack", [NBLK, 128, NPACK, BLK], f16, kind="ExternalInput").ap()
    m678_d = nc.dram_tensor("M678", [NSLOT, 128, 1, BLK], f16, kind="ExternalInput").ap()
    # weights in two host-laid-out fp16 blobs (W1 separate so the first
    # LDWEIGHTS isn't gated on the full blob), biases in one fp32 blob
    wb1_d = nc.dram_tensor("WB1", [128, 256], f16, kind="ExternalInput").ap()
    wb_d = nc.dram_tensor("WB", [128, 2688], f16, kind="ExternalInput").ap()
    bb_d = nc.dram_tensor("BB", [128, 12], f32, kind="ExternalInput").ap()
    out_d = nc.dram_tensor("outT", [10, SHARD], f32, kind="ExternalOutput").ap()

    with tile.TileContext(nc) as tc:
        with (
            tc.tile_pool(name="wpool", bufs=1) as wp,
            tc.tile_pool(name="mk", bufs=2) as mkp,
            tc.tile_pool(name="hr", bufs=1) as hrp,
            tc.tile_pool(name="osb", bufs=2) as outp,
            tc.tile_pool(name="ps", bufs=4, space="PSUM") as psp,
        ):
            wall = wp.tile([128, 2944], f16, tag="wall")
            ball = wp.tile([128, 12], f32, tag="ball")
            scratch = wp.tile([128, 512], f16, tag="scratch")
            # blob column offsets: w1@0(256) w2@256(256) w3@512(1024) w4@1536(1024)
            #   w5@2560(256) w6@2816(64) w789@2880(64: W7 r0-63 c0-31, W8 r64-95
            #   c32-47, W9 r96-111 c48-57)
            WOFF = {1: 0, 2: 256, 3: 512, 4: 1536, 5: 2560, 6: 2816, 789: 2880}
            w789 = wall[:, WOFF[789]:WOFF[789] + 64]
            b15 = ball[:, 0:10]
            b678 = ball[:, 10:11]
            b9 = ball[0:10, 11:12]

            def wslice(l, k, c, N):
                base = WOFF[l] + k * N
                return wall[:, base + c * 128: base + (c + 1) * 128]

            def drain_relu(eng, dst, zsrc, bias_ap):
                if eng == "act":
                    nc.scalar.activation(dst, zsrc, RELU, bias=bias_ap)
                else:
                    nc.vector.tensor_scalar(dst, zsrc, bias_ap, 0.0, ADD, MAX)

            def mask_mul(eng, dst, src, msrc):
                if eng == "gps":
                    nc.gpsimd.tensor_mul(dst, src, msrc)
                else:
                    nc.vector.tensor_mul(dst, src, msrc)

            # drain engine picker: ~70:30 act:dve (ACT is cheaper per element but
            # DVE has mask work too; this balances their queues)
            dr_i = [0]

            def pick_drain():
                i = dr_i[0]
                dr_i[0] += 1
                return "dve" if i % 10 in (2, 5, 8) else "act"

            state = {}
            packs = {}
            hrs = {}

            def issue_pack_dmas(b):
                # per-chunk tiles/DMAs: each mask tile's ring slot is released as
                # soon as its own layer consumes it, so block b+2's DMAs start
                # early instead of waiting for ALL of block b's masks (WAR).
                pk3 = mkp.tile([128, 3, BLK], f16, tag="pk3", name=f"pk3_{b}")
                m2t = mkp.tile([128, 2, BLK], f16, tag="m2", name=f"m2_{b}")
                m3t = mkp.tile([128, 4, BLK], f16, tag="m3", name=f"m3_{b}")
                m4t = mkp.tile([128, 2, BLK], f16, tag="m4", name=f"m4_{b}")
                m5t = mkp.tile([128, 1, BLK], f16, tag="m5", name=f"m5_{b}")
                if b == 0:
                    nc.sync.dma_start(wall[:, 0:256], wb1_d[:])
                    nc.sync.dma_start(ball[:], bb_d[:])
                    for q in range(NSUB):
                        qs = bass.ts(q, MSUB)
                        nc.sync.dma_start(pk3[:, :, qs], pack_d[0, :, 0:3, qs])
                    nc.sync.dma_start(wall[:, 256:], wb_d[:])
                else:
                    nc.sync.dma_start(pk3[:], pack_d[b, :, 0:3, :])
                nc.sync.dma_start(m2t[:], pack_d[b, :, 3:5, :])
                nc.sync.dma_start(m3t[:], pack_d[b, :, 5:9, :])
                nc.sync.dma_start(m4t[:], pack_d[b, :, 9:11, :])
                nc.sync.dma_start(m5t[:], pack_d[b, :, 11:12, :])
                packs[b] = (pk3, m2t, m3t, m4t, m5t)
                issue_m678_dma(b)

            m678s = {}

            def issue_m678_dma(i):
                m678s[i] = mkp.tile([128, 1, BLK], f16, tag="m678", name=f"m678s_{i}",
                                    bufs=3)
                nc.sync.dma_start(m678s[i][:], m678_d[i])

            # (Kc, layer, wN, Cc, bias_off, hrtag)
            LAYER_CFG = [
                (2, 1, 128, 1, 0, "hr1"),
                (1, 2, 256, 2, 1, "hr2"),
                (2, 3, 512, 4, 3, "hr3"),
                (4, 4, 256, 2, 7, "hr4"),
                (2, 5, 128, 1, 9, "hr5"),
            ]
            # mask engine schedule per layer index. GpSimd is ~4x slower than DVE
            # per element, so it only gets masks with slack before their consumer:
            # m1 (L1 runs a block ahead) and m5 (ladder consumes it next block).
            # L2/L3/L4 masks sit on the next layer's critical path -> DVE only.
            MASK_MODE = {
                0: ["dve2"],
                1: ["dve2", "dve2"],
                2: ["dve2", "dve2", "dve2", "dve2"],
                3: ["dve2", "dve2"],
                4: ["dve2"],
            }
            # per-layer drain engine assignment, in (u, c) order. ACT-heavy for
            # the mid-block bulk (L3/L4); the boundary-critical L2 keeps one on
            # DVE so its chain isn't stuck behind the ACT queue.
            DRAIN_ENG = {
                0: ["dve", "act"],
                1: ["act", "act", "dve", "act"],
                2: ["act", "act", "act", "dve", "act", "act", "act", "act"],
                3: ["act", "act", "act", "act"],
                4: ["dve", "act"],
            }

            def emit_layer(b, li):
                Kc, wl, wN, Cc, boff, hrtag = LAYER_CFG[li]
                hr = hrp.tile([128, Cc, BLK], f16, tag=hrtag, name=hrtag + f"_{b}",
                              bufs=2 if hrtag in ("hr5", "hr2", "hr1") else 1)
                pk3 = packs[b][0]
                hin = pk3 if li == 0 else hrs[(b, li - 1)]

                def msl(c, cols):
                    if li == 0:
                        return pk3[:, 2, cols]
                    return packs[b][li][:, c, cols]

                zs = {}
                for c in range(Cc):
                    for u in range(NDR):
                        zs[c, u] = psp.tile([128, DSUB], f32, tag="ps",
                                            name=f"z_{hrtag}_{b}_{c}_{u}")
                if b == 0 and li == 0:
                    # startup: 512-col windows so the first matmul fires as soon
                    # as the first x quarter lands
                    for t in range(NSUB):
                        ts_ = bass.ts(t, MSUB)
                        for k in range(Kc):
                            nc.tensor.matmul(
                                zs[0, t // 2][:, bass.ts(t % 2, MSUB)],
                                wslice(wl, k, 0, wN), hin[:, k, ts_],
                                start=(k == 0), stop=(k == Kc - 1))
                        drain_relu("dve" if t % 2 == 0 else "act",
                                   hr[:, 0, ts_], zs[0, t // 2][:, bass.ts(t % 2, MSUB)],
                                   b15[:, 0:1])
                        mask_mul("dve", hr[:, 0, ts_], hr[:, 0, ts_], msl(0, ts_))
                    hrs[(b, li)] = hr
                    return
                # weight-major matmuls so consecutive MMs share one LDWEIGHTS
                for c in range(Cc):
                    for k in range(Kc):
                        wap = wslice(wl, k, c, wN)
                        for t in range(NSUB):
                            nc.tensor.matmul(
                                zs[c, t // 2][:, bass.ts(t % 2, MSUB)], wap,
                                hin[:, k, bass.ts(t, MSUB)],
                                start=(k == 0), stop=(k == Kc - 1))
                de = DRAIN_ENG[li]
                for u in range(NDR):
                    for c in range(Cc):
                        drain_relu(de[u * Cc + c], hr[:, c, bass.ts(u, DSUB)],
                                   zs[c, u][:], b15[:, boff + c:boff + c + 1])
                full = slice(0, BLK)
                for c in range(Cc):
                    mode = MASK_MODE[li][c]
                    if mode == "dve":
                        mask_mul("dve", hr[:, c, full], hr[:, c, full], msl(c, full))
                    elif mode == "dve2":
                        for uu in range(NDR):
                            hs = bass.ts(uu, DSUB)
                            mask_mul("dve", hr[:, c, hs], hr[:, c, hs], msl(c, hs))
                    else:
                        hs0, hs1 = bass.ts(0, DSUB), bass.ts(1, DSUB)
                        mask_mul("dve", hr[:, c, hs0], hr[:, c, hs0], msl(c, hs0))
                        mask_mul("gps", hr[:, c, hs1], hr[:, c, hs1], msl(c, hs1))
                hrs[(b, li)] = hr
                if li > 0:
                    del hrs[(b, li - 1)]

            # --- small-layer ladder, slot-fused across blocks --------------------
            # Slot i co-issues three INDEPENDENT small matmuls from staggered
            # blocks into disjoint partition ranges of ONE PSUM tile:
            #   rows 0:64   L6(block i)    cols 0:64   of the PE array
            #   rows 64:96  L7(block i-1)  cols 64:96  (tile_position (0,64))
            #   rows 96:112 L8(block i-2)  cols 96:128 (tile_position (64,96))
            # They run concurrently (disjoint subarrays), and ONE drain + ONE
            # mask serves all three (the host staggers the m678 mask planes the
            # same way). L9(block i-2) then reads rows 96:112 after the mask.
            slots = {}
            hm5s = {}

            def slot_wins(fine):
                return (range(NSUB), MSUB) if fine else (range(NDR), DSUB)

            def emit_slot_trio(i, fine=False):
                sl = slots.setdefault(i, {})
                sl["hr678"] = hrp.tile([128, 1, BLK], f16, tag="hr678",
                                       name=f"hr678_{i}", bufs=2)
                prev = slots.get(i - 1)
                wins, wsz = slot_wins(fine)
                nmm = wsz // MSUB
                zhs = {}
                for u in wins:
                    zhs[u] = psp.tile([128, wsz], f32, tag="ps", name=f"zh_{i}_{u}")
                sl["zhs"] = zhs
                sl["fine"] = fine
                # which small layers live in slot i: slots 0..2 accumulate the
                # stagger; slots 3 (L7+L8) and 4 (L8) run inside block 3; block
                # 3's own ladder runs v3-style in the tail.
                PARTS = {0: (6,), 1: (6, 7), 2: (6, 7, 8), 3: (7, 8), 4: (8,)}
                parts = []
                if 6 in PARTS[i]:
                    parts.append((0, 64, wall[:, WOFF[6]:WOFF[6] + 64], None,
                                  lambda cs: hm5s[i][:, 0, cs]))
                if 7 in PARTS[i]:
                    parts.append((64, 96, w789[0:64, 0:32], (0, 64),
                                  lambda cs: prev["hr678"][0:64, 0, cs]))
                if 8 in PARTS[i]:
                    parts.append((96, 112, w789[64:96, 32:48], (64, 96),
                                  lambda cs: prev["hr678"][64:96, 0, cs]))
                sl["p_lo"] = parts[0][0]
                sl["p_hi"] = parts[-1][1]
                # weight-major: one LDW per weight per slot (consecutive dedup),
                # streams of different col-groups overlap on the PE
                for p0, p1, wap, tpos, rhs_of in parts:
                    for u in wins:
                        for t in range(nmm):
                            cs = slice(u * wsz + t * MSUB, u * wsz + (t + 1) * MSUB)
                            out_sl = zhs[u][p0:p1, bass.ts(t, MSUB)]
                            if tpos is None:
                                nc.tensor.matmul(out_sl, wap, rhs_of(cs),
                                                 start=True, stop=True)
                            else:
                                nc.tensor.matmul(out_sl, wap, rhs_of(cs),
                                                 start=True, stop=True,
                                                 tile_position=tpos)
                if i in hm5s:
                    del hm5s[i]

            def emit_slot_drainmask(i):
                sl = slots[i]
                p_lo, p_hi = sl["p_lo"], sl["p_hi"]
                hr678 = sl["hr678"]
                m678 = m678s[i]
                wins, wsz = slot_wins(sl["fine"])
                for u in wins:
                    dst = hr678[p_lo:p_hi, 0, u * wsz:(u + 1) * wsz]
                    drain_relu("dve" if u % 2 == 0 else "act", dst,
                               sl["zhs"][u][p_lo:p_hi, :], b678[p_lo:p_hi, 0:1])
                    # hs1 half -> gps only at coarse granularity (slack: consumers
                    # are a block away except L9, which is ~4us later)
                    mask_mul("gps" if (not sl["fine"] and u == 1) else "dve",
                             dst, dst, m678[p_lo:p_hi, 0, u * wsz:(u + 1) * wsz])
                del sl["zhs"]

            def emit_slot_l9(i):
                b = i - 2
                sl = slots[i]
                hm678 = sl["hr678"]
                fine = sl["fine"]
                osb = outp.tile([10, BLK], f32, tag="osb", bufs=2, name=f"osb_{b}")
                wins, wsz = slot_wins(fine)
                nmm = wsz // MSUB
                for u in wins:
                    z9 = psp.tile([128, wsz], f32, tag="ps", name=f"z9_{b}_{u}")
                    for t in range(nmm):
                        rhs_sl = slice(u * wsz + t * MSUB, u * wsz + (t + 1) * MSUB)
                        nc.tensor.matmul(z9[0:10, bass.ts(t, MSUB)],
                                         w789[96:112, 48:58],
                                         hm678[96:112, 0, rhs_sl],
                                         start=True, stop=True, tile_position=(96, 0))
                    if u % 2 == 0:
                        nc.scalar.activation(osb[:, u * wsz:(u + 1) * wsz],
                                             z9[0:10, :], IDENT, bias=b9[:, 0:1])
                    else:
                        nc.vector.tensor_scalar(osb[:, u * wsz:(u + 1) * wsz],
                                                z9[0:10, :], b9[:, 0:1], None, ADD)
                    if (u + 1) * wsz == DSUB or (u + 1) * wsz == BLK:
                        h0 = (u + 1) * wsz - DSUB
                        nc.sync.dma_start(
                            out_d[:, b * BLK + h0:b * BLK + h0 + DSUB],
                            osb[:, h0:h0 + DSUB])
                if (i - 1) in slots:
                    del slots[i - 1]

            # --- schedule ---------------------------------------------------------
            # PE warmup: dummy matmuls on a memset scratch tile so the HAM clock
            # gate opens before real work arrives (weights/x still in DMA).
            nc.vector.memset(scratch[:], 0.0)
            zw = psp.tile([128, MSUB], f32, tag="ps", name="zwarm")
            for i in range(10):
                nc.tensor.matmul(zw[:], scratch[:, 0:128], scratch[:], start=True,
                                 stop=True)

            def warm(n):
                # filler matmuls that keep the PE HAM activity window busy while
                # a dependency chain stalls the real stream (tail)
                zf = psp.tile([128, MSUB], f32, tag="ps", name="zf")
                for _ in range(n):
                    nc.tensor.matmul(zf[:], scratch[:, 0:128], scratch[:],
                                     start=True, stop=True)

            issue_pack_dmas(0)
            emit_layer(0, 0)                       # L1(0) during startup
            for b in range(NBLK):
                if b + 1 < NBLK:
                    issue_pack_dmas(b + 1)
                if b == 2:
                    issue_m678_dma(4)              # phantom tail-slot planes
                if b == 3:
                    issue_m678_dma(5)
                if b >= 1:
                    warm(3)                        # bridge the block handover
                emit_layer(b, 1)                   # L2
                if b >= 1:
                    emit_slot_trio(b - 1)          # L6(b-1)+L7(b-2)+L8(b-3)
                emit_layer(b, 2)                   # L3
                if b >= 1:
                    emit_slot_drainmask(b - 1)
                emit_layer(b, 3)                   # L4
                if b == NBLK - 1:
                    emit_slot_trio(3)              # L7(2)+L8(1): no block-3 deps
                    emit_slot_drainmask(3)
                if b + 1 < NBLK:
                    emit_layer(b + 1, 0)           # L1(b+1) pipelined ahead
                if b >= 1 and b - 1 >= 2:
                    emit_slot_l9(b - 1)            # out block 0
                emit_layer(b, 4)                   # L5
                hm5s[b] = hrs.pop((b, 4))

            # tail: only block 3's own ladder (v3-style, fine windows) remains on
            # the critical chain; the independent slot-4/l9 work and warm()
            # fillers run during each ladder step's drain+mask latency.
            lad = hrp.tile([128, 1, BLK], f16, tag="hr678", name="lad3", bufs=2)
            m6d = m678s[5]                         # diagonal plane {m6,m7,m8}(3)
            LADW = [(0, 64, wall[:, WOFF[6]:WOFF[6] + 64], None, None),
                    (64, 96, w789[0:64, 0:32], (0, 64), (0, 64)),
                    (96, 112, w789[64:96, 32:48], (64, 96), (64, 96))]

            def lad_step(step):
                p0, p1, wap, tpos, brg = LADW[step]
                for w in range(NSUB):
                    zh = psp.tile([128, MSUB], f32, tag="ps", name=f"lz_{step}_{w}")
                    cs = slice(w * MSUB, (w + 1) * MSUB)
                    rhs = (hm5s[3][:, 0, cs] if step == 0 else
                           lad[brg[0]:brg[1], 0, cs])
                    if tpos is None:
                        nc.tensor.matmul(zh[p0:p1, :], wap, rhs, start=True,
                                         stop=True)
                    else:
                        nc.tensor.matmul(zh[p0:p1, :], wap, rhs, start=True,
                                         stop=True, tile_position=tpos)
                    dst = lad[p0:p1, 0, cs]
                    drain_relu("dve" if w % 2 == 0 else "act", dst, zh[p0:p1, :],
                               b678[p0:p1, 0:1])
                    mask_mul("dve", dst, dst, m6d[p0:p1, 0, cs])

            emit_slot_l9(3)                        # out block 1 (fills L5 drain wait)
            lad_step(0)                            # L6(3)
            emit_slot_trio(4)                      # L8(2): independent, fills wait
            emit_slot_drainmask(4)
            lad_step(1)                            # L7(3)
            emit_slot_l9(4)                        # out block 2: fills wait
            lad_step(2)                            # L8(3)
            warm(5)
            slots[5] = {"hr678": lad, "fine": True, "p_lo": 0, "p_hi": 112}
            emit_slot_l9(5)                        # out block 3

    _dedup_ldweights(nc)
    nc.compile()
    return nc


def _get_program():
    if "nc" not in _PROG:
        _PROG["nc"] = _build_program()
    return _PROG["nc"]


def _host_prep(inputs):
    """Build per-core input maps (numpy only)."""
    x = np.asarray(inputs["x"], dtype=np.float32)
    Ws = [np.asarray(inputs[f"W{i}"], dtype=np.float32) for i in range(1, 10)]
    bs = [np.asarray(inputs[f"b{i}"], dtype=np.float32) for i in range(1, 10)]
    ms = [np.asarray(inputs[f"m{i}"], dtype=np.float32) for i in range(1, 9)]

    # fold dropout scale into next layer's weights; binarize masks
    Wf = [Ws[0]]
    for i in range(1, 9):
        s = float(ms[i - 1].max())
        if s <= 0.0:  # degenerate all-dropped mask; keep weights unscaled
            s = 1.0
        Wf.append(Ws[i] * np.float32(s))

    # weight blob: w1@0 w2@256 w3@512 w4@1536 w5@2560 w6@2816 w789@2880
    WOFF = {1: 0, 2: 256, 3: 512, 4: 1536, 5: 2560, 6: 2816, 789: 2880}
    wb = np.zeros((128, 2944), dtype=np.float16)
    for l in range(1, 7):
        W = Wf[l - 1]
        K, N = W.shape
        for k in range((K + 127) // 128):
            blk = W[k * 128:(k + 1) * 128].astype(np.float16)
            wb[: blk.shape[0], WOFF[l] + k * N: WOFF[l] + k * N + N] = blk
    wb[0:64, 2880:2912] = Wf[6].astype(np.float16)    # W7
    wb[64:96, 2912:2928] = Wf[7].astype(np.float16)   # W8
    wb[96:112, 2928:2938] = Wf[8].astype(np.float16)  # W9
    wb1, wb = np.ascontiguousarray(wb[:, 0:256]), np.ascontiguousarray(wb[:, 256:])
    bb = np.zeros((128, 12), dtype=np.float32)
    bb[:, 0] = bs[0]
    bb[:, 1], bb[:, 2] = bs[1][0:128], bs[1][128:256]
    for c in range(4):
        bb[:, 3 + c] = bs[2][c * 128:(c + 1) * 128]
    bb[:, 7], bb[:, 8] = bs[3][0:128], bs[3][128:256]
    bb[:, 9] = bs[4]
    bb[0:64, 10], bb[64:96, 10], bb[96:112, 10] = bs[5], bs[6], bs[7]
    bb[0:10, 11] = bs[8]
    shared = {"WB1": wb1, "WB": wb, "BB": bb}

    in_maps = []
    for c in range(NCORES):
        sl = slice(c * SHARD, (c + 1) * SHARD)
        pack = np.zeros((NBLK, 128, NPACK, BLK), dtype=np.float16)
        m678 = np.zeros((NSLOT, 128, 1, BLK), dtype=np.float16)
        xT = x[sl].T  # (256, SHARD)
        mT = [None] + [(ms[i][sl] != 0).T.astype(np.float16) for i in range(8)]
        for b in range(NBLK):
            cs = slice(b * BLK, (b + 1) * BLK)
            pack[b, :, 0, :] = xT[0:128, cs]
            pack[b, :, 1, :] = xT[128:256, cs]
            pack[b, :, 2, :] = mT[1][:, cs]
            pack[b, :, 3, :], pack[b, :, 4, :] = mT[2][0:128, cs], mT[2][128:256, cs]
            for k in range(4):
                pack[b, :, 5 + k, :] = mT[3][k * 128:(k + 1) * 128, cs]
            pack[b, :, 9, :], pack[b, :, 10, :] = mT[4][0:128, cs], mT[4][128:256, cs]
            pack[b, :, 11, :] = mT[5][:, cs]
        # slot-staggered m678 planes: slot i masks {m6(i), m7(i-1), m8(i-2)};
        # plane 5 is block 3's diagonal {m6(3), m7(3), m8(3)} for the tail ladder
        for i in range(NSLOT - 1):
            if i < NBLK:
                m678[i, 0:64, 0, :] = mT[6][:, i * BLK:(i + 1) * BLK]
            if 0 <= i - 1 < NBLK:
                m678[i, 64:96, 0, :] = mT[7][:, (i - 1) * BLK:i * BLK]
            if 0 <= i - 2 < NBLK:
                m678[i, 96:112, 0, :] = mT[8][:, (i - 2) * BLK:(i - 1) * BLK]
        lb = slice((NBLK - 1) * BLK, NBLK * BLK)
        m678[5, 0:64, 0, :] = mT[6][:, lb]
        m678[5, 64:96, 0, :] = mT[7][:, lb]
        m678[5, 96:112, 0, :] = mT[8][:, lb]
        in_maps.append({"pack": pack, "M678": m678, **shared})
    return in_maps


def kernel(**inputs) -> np.ndarray:
    from concourse.bass_utils import run_bass_kernel_spmd

    nc = _get_program()
    in_maps = _host_prep(inputs)
    res = run_bass_kernel_spmd(nc, in_maps, list(range(NCORES)))
    out = np.empty((BATCH, DIMS[-1]), dtype=np.float32)
    for c in range(NCORES):
        out[c * SHARD:(c + 1) * SHARD, :] = res.results[c]["outT"].T
    return out


# revision 24
# speedup vs baseline: 1.2469x; 1.0161x over previous
"""Trainium2 Bass kernel for the 9-layer dense MLP (dropout-mask training forward).

Strategy (pure data parallel, 8 cores, 8192 batch rows each):
  - Activations kept transposed on-chip: features on partitions, batch cols on free dim.
    Each layer computes zT = W^T @ hT via nc.tensor.matmul(out, lhsT=W, rhs=hT).
  - fp16 weights/activations/masks (fp32 PSUM accumulation), fp32 biases + output.
  - Dropout masks binarized on host ({0,1} fp16); the 1/keep scale is folded into the
    next layer's weights.
  - Host pack layout [NBLK, 128, NPACK, BLK] so each per-block DMA is contiguous per
    partition (2 DMAs per block: x+m1, then the remaining masks).
  - PSUM: one shared pool of 4x [128,1024] fp32 tiles (8 banks). Matmuls write 512-col
    halves; drains are single FD=1024 instructions (fused bias+relu) split ~3:1
    ACT:DVE; mask multiplies are FD=2048 DVE tensor_tensor with a measured dose
    offloaded to GpSimd.
  - Small layers 6/7/8 partition-packed (offsets 0/64/96 via matmul tile_position);
    each ladder step drains immediately to SBUF so its PSUM tile recycles fast.
    Block b's ladder is software-pipelined into block b+1's big-layer bursts.
  - A short burst of dummy matmuls at t~1us keeps the PE HAM activity monitor busy so
    the array is at full clock (K=8/8) when real work arrives.
"""

import sys

sys.path.insert(0, "/opt/trn_rl_repo")

import numpy as np

DIMS = [256, 128, 256, 512, 256, 128, 64, 32, 16, 10]
NCORES = 8
BATCH = 65536
SHARD = BATCH // NCORES  # 8192
MSUB = 512               # matmul N (PSUM bank limit for fp32)
DSUB = 1024              # drain granularity (2 banks)
BLK = 2048               # block columns
NBLK = SHARD // BLK      # 4
NSUB = BLK // MSUB       # 4
NDR = BLK // DSUB        # 2

# pack chunk layout (each chunk = 128 partitions x BLK cols, fp16), per block:
#   0,1: xT   2: m1   3,4: m2   5-8: m3   9,10: m4   11: m5
# m678 ships separately as NBLK+2 slot-staggered planes (rows 0:64 = m6(slot),
# 64:96 = m7(slot-1), 96:112 = m8(slot-2)) to match the slot-fused ladder.
NPACK = 12
NSLOT = NBLK + 2

_PROG = {}


def _raise_sbuf_cap():
    # tile_utils.max_sbuf_usage is a stale 192KB constant; cayman has 208KB usable.
    import concourse.tile_utils as tu

    if getattr(tu, "max_sbuf_usage", 0) < 206 * 1024:
        tu.max_sbuf_usage = 206 * 1024


def _dedup_ldweights(nc):
    """Remove back-to-back redundant LDWEIGHTS (same stationary operand) so
    consecutive same-weight matmuls pipeline on the PE. Only drops LDW
    instructions that carry no semaphore waits/updates."""
    removed = 0
    for fn in nc.m.functions:
        for blk in fn.blocks:
            il = blk.instructions
            keep, last_sig = [], None
            for inst in il:
                nm = type(inst).__name__
                if nm == "InstLdweights":
                    sig = (str(inst.ins[0]), str(inst.is_transpose), str(inst.perf_mode),
                           str(getattr(inst, "tile_position", None)))
                    si = inst.sync_info
                    clean = si is None or (not si.on_wait and not si.on_update)
                    if sig == last_sig and clean:
                        removed += 1
                        continue
                    last_sig = sig
                keep.append(inst)
            if removed and len(keep) != len(il):
                while il:
                    il.pop()
                il.extend(keep)
    return removed


def _build_program():
    import concourse.bass as bass
    import concourse.tile as tile
    from concourse import bacc, mybir

    _raise_sbuf_cap()

    f16 = mybir.dt.float16
    f32 = mybir.dt.float32
    RELU = mybir.ActivationFunctionType.Relu
    IDENT = mybir.ActivationFunctionType.Identity
    ADD = mybir.AluOpType.add
    MAX = mybir.AluOpType.max

    nc = bacc.Bacc("TRN2", target_bir_lowering=False, debug=False, num_devices=NCORES)

    pack_d = nc.dram_tensor("pack", [NBLK, 128, NPACK, BLK], f16, kind="ExternalInput").ap()
    m678_d = nc.dram_tensor("M678", [NSLOT, 128, 1, BLK], f16, kind="ExternalInput").ap()
    # weights in two host-laid-out fp16 blobs (W1 separate so the first
    # LDWEIGHTS isn't gated on the full blob), biases in one fp32 blob
    wb1_d = nc.dram_tensor("WB1", [128, 256], f16, kind="ExternalInput").ap()
    wb_d = nc.dram_tensor("WB", [128, 2688], f16, kind="ExternalInput").ap()
    bb_d = nc.dram_tensor("BB", [128, 12], f32, kind="ExternalInput").ap()
    out_d = nc.dram_tensor("outT", [10, SHARD], f32, kind="ExternalOutput").ap()

    with tile.TileContext(nc) as tc:
        with (
            tc.tile_pool(name="wpool", bufs=1) as wp,
            tc.tile_pool(name="mk", bufs=2) as mkp,
            tc.tile_pool(name="hr", bufs=1) as hrp,
            tc.tile_pool(name="osb", bufs=2) as outp,
            tc.tile_pool(name="ps", bufs=4, space="PSUM") as psp,
        ):
            wall = wp.tile([128, 2944], f16, tag="wall")
            ball = wp.tile([128, 12], f32, tag="ball")
            scratch = wp.tile([128, 512], f16, tag="scratch")
            # blob column offsets: w1@0(256) w2@256(256) w3@512(1024) w4@1536(1024)
            #   w5@2560(256) w6@2816(64) w789@2880(64: W7 r0-63 c0-31, W8 r64-95
            #   c32-47, W9 r96-111 c48-57)
            WOFF = {1: 0, 2: 256, 3: 512, 4: 1536, 5: 2560, 6: 2816, 789: 2880}
            w789 = wall[:, WOFF[789]:WOFF[789] + 64]
            b15 = ball[:, 0:10]
            b678 = ball[:, 10:11]
            b9 = ball[0:10, 11:12]

            def wslice(l, k, c, N):
                base = WOFF[l] + k * N
                return wall[:, base + c * 128: base + (c + 1) * 128]

            def drain_relu(eng, dst, zsrc, bias_ap):
                if eng == "act":
                    nc.scalar.activation(dst, zsrc, RELU, bias=bias_ap)
                else:
                    nc.vector.tensor_scalar(dst, zsrc, bias_ap, 0.0, ADD, MAX)

            def mask_mul(eng, dst, src, msrc):
                if eng == "gps":
                    nc.gpsimd.tensor_mul(dst, src, msrc)
                else:
                    nc.vector.tensor_mul(dst, src, msrc)

            # drain engine picker: ~70:30 act:dve (ACT is cheaper per element but
            # DVE has mask work too; this balances their queues)
            dr_i = [0]

            def pick_drain():
                i = dr_i[0]
                dr_i[0] += 1
                return "dve" if i % 10 in (2, 5, 8) else "act"

            state = {}
            packs = {}
            hrs = {}

            def issue_pack_dmas(b):
                # per-chunk tiles/DMAs: each mask tile's ring slot is released as
                # soon as its own layer consumes it, so block b+2's DMAs start
                # early instead of waiting for ALL of block b's masks (WAR).
                pk3 = mkp.tile([128, 3, BLK], f16, tag="pk3", name=f"pk3_{b}")
                m2t = mkp.tile([128, 2, BLK], f16, tag="m2", name=f"m2_{b}")
                m3t = mkp.tile([128, 4, BLK], f16, tag="m3", name=f"m3_{b}")
                m4t = mkp.tile([128, 2, BLK], f16, tag="m4", name=f"m4_{b}")
                m5t = mkp.tile([128, 1, BLK], f16, tag="m5", name=f"m5_{b}")
                if b == 0:
                    nc.sync.dma_start(wall[:, 0:256], wb1_d[:])
                    nc.sync.dma_start(ball[:], bb_d[:])
                    for q in range(NSUB):
                        qs = bass.ts(q, MSUB)
                        nc.sync.dma_start(pk3[:, :, qs], pack_d[0, :, 0:3, qs])
                    nc.sync.dma_start(wall[:, 256:], wb_d[:])
                else:
                    nc.sync.dma_start(pk3[:], pack_d[b, :, 0:3, :])
                nc.sync.dma_start(m2t[:], pack_d[b, :, 3:5, :])
                nc.sync.dma_start(m3t[:], pack_d[b, :, 5:9, :])
                nc.sync.dma_start(m4t[:], pack_d[b, :, 9:11, :])
                nc.sync.dma_start(m5t[:], pack_d[b, :, 11:12, :])
                packs[b] = (pk3, m2t, m3t, m4t, m5t)
                issue_m678_dma(b)

            m678s = {}

            def issue_m678_dma(i):
                m678s[i] = mkp.tile([128, 1, BLK], f16, tag="m678", name=f"m678s_{i}",
                                    bufs=3)
                nc.sync.dma_start(m678s[i][:], m678_d[i])

            # (Kc, layer, wN, Cc, bias_off, hrtag)
            LAYER_CFG = [
                (2, 1, 128, 1, 0, "hr1"),
                (1, 2, 256, 2, 1, "hr2"),
                (2, 3, 512, 4, 3, "hr3"),
                (4, 4, 256, 2, 7, "hr4"),
                (2, 5, 128, 1, 9, "hr5"),
            ]
            # mask engine schedule per layer index. GpSimd is ~4x slower than DVE
            # per element, so it only gets masks with slack before their consumer:
            # m1 (L1 runs a block ahead) and m5 (ladder consumes it next block).
            # L2/L3/L4 masks sit on the next layer's critical path -> DVE only.
            MASK_MODE = {
                0: ["dve2"],
                1: ["dve2", "dve2"],
                2: ["dve2", "dve2", "dve2", "dve2"],
                3: ["dve2", "dve2"],
                4: ["dve2"],
            }
            # per-layer drain engine assignment, in (u, c) order. ACT-heavy for
            # the mid-block bulk (L3/L4); the boundary-critical L2 keeps one on
            # DVE so its chain isn't stuck behind the ACT queue.
            DRAIN_ENG = {
                0: ["dve", "act"],
                1: ["act", "act", "dve", "act"],
                2: ["act", "act", "act", "dve", "act", "act", "act", "act"],
                3: ["act", "act", "act", "act"],
                4: ["dve", "act"],
            }

            def emit_layer(b, li):
                Kc, wl, wN, Cc, boff, hrtag = LAYER_CFG[li]
                hr = hrp.tile([128, Cc, BLK], f16, tag=hrtag, name=hrtag + f"_{b}",
                              bufs=2 if hrtag in ("hr5", "hr2", "hr1") else 1)
                pk3 = packs[b][0]
                hin = pk3 if li == 0 else hrs[(b, li - 1)]

                def msl(c, cols):
                    if li == 0:
                        return pk3[:, 2, cols]
                    return packs[b][li][:, c, cols]

                zs = {}
                for c in range(Cc):
                    for u in range(NDR):
                        zs[c, u] = psp.tile([128, DSUB], f32, tag="ps",
                                            name=f"z_{hrtag}_{b}_{c}_{u}")
                if b == 0 and li == 0:
                    # startup: 512-col windows so the first matmul fires as soon
                    # as the first x quarter lands
                    for t in range(NSUB):
                        ts_ = bass.ts(t, MSUB)
                        for k in range(Kc):
                            nc.tensor.matmul(
                                zs[0, t // 2][:, bass.ts(t % 2, MSUB)],
                                wslice(wl, k, 0, wN), hin[:, k, ts_],
                                start=(k == 0), stop=(k == Kc - 1))
                        drain_relu("dve" if t % 2 == 0 else "act",
                                   hr[:, 0, ts_], zs[0, t // 2][:, bass.ts(t % 2, MSUB)],
                                   b15[:, 0:1])
                        mask_mul("dve", hr[:, 0, ts_], hr[:, 0, ts_], msl(0, ts_))
                    hrs[(b, li)] = hr
                    return
                # weight-major matmuls so consecutive MMs share one LDWEIGHTS
                for c in range(Cc):
                    for k in range(Kc):
                        wap = wslice(wl, k, c, wN)
                        for t in range(NSUB):
                            nc.tensor.matmul(
                                zs[c, t // 2][:, bass.ts(t % 2, MSUB)], wap,
                                hin[:, k, bass.ts(t, MSUB)],
                                start=(k == 0), stop=(k == Kc - 1))
                de = DRAIN_ENG[li]
                for u in range(NDR):
                    for c in range(Cc):
                        drain_relu(de[u * Cc + c], hr[:, c, bass.ts(u, DSUB)],
                                   zs[c, u][:], b15[:, boff + c:boff + c + 1])
                full = slice(0, BLK)
                for c in range(Cc):
                    mode = MASK_MODE[li][c]
                    if mode == "dve":
                        mask_mul("dve", hr[:, c, full], hr[:, c, full], msl(c, full))
                    elif mode == "dve2":
                        for uu in range(NDR):
                            hs = bass.ts(uu, DSUB)
                            mask_mul("dve", hr[:, c, hs], hr[:, c, hs], msl(c, hs))
                    else:
                        hs0, hs1 = bass.ts(0, DSUB), bass.ts(1, DSUB)
                        mask_mul("dve", hr[:, c, hs0], hr[:, c, hs0], msl(c, hs0))
                        mask_mul("gps", hr[:, c, hs1], hr[:, c, hs1], msl(c, hs1))
                hrs[(b, li)] = hr
                if li > 0:
                    del hrs[(b, li - 1)]

            # --- small-layer ladder, slot-fused across blocks --------------------
            # Slot i co-issues three INDEPENDENT small matmuls from staggered
            # blocks into disjoint partition ranges of ONE PSUM tile:
            #   rows 0:64   L6(block i)    cols 0:64   of the PE array
            #   rows 64:96  L7(block i-1)  cols 64:96  (tile_position (0,64))
            #   rows 96:112 L8(block i-2)  cols 96:128 (tile_position (64,96))
            # They run concurrently (disjoint subarrays), and ONE drain + ONE
            # mask serves all three (the host staggers the m678 mask planes the
            # same way). L9(block i-2) then reads rows 96:112 after the mask.
            slots = {}
            hm5s = {}

            def slot_wins(fine):
                return (range(NSUB), MSUB) if fine else (range(NDR), DSUB)

            def emit_slot_trio(i, fine=False):
                sl = slots.setdefault(i, {})
                sl["hr678"] = hrp.tile([128, 1, BLK], f16, tag="hr678",
                                       name=f"hr678_{i}", bufs=2)
                prev = slots.get(i - 1)
                wins, wsz = slot_wins(fine)
                nmm = wsz // MSUB
                zhs = {}
                for u in wins:
                    zhs[u] = psp.tile([128, wsz], f32, tag="ps", name=f"zh_{i}_{u}")
                sl["zhs"] = zhs
                sl["fine"] = fine
                # which small layers live in slot i: slots 0..2 accumulate the
                # stagger; slots 3 (L7+L8) and 4 (L8) run inside block 3; block
                # 3's own ladder runs v3-style in the tail.
                PARTS = {0: (6,), 1: (6, 7), 2: (6, 7, 8), 3: (7, 8), 4: (8,)}
                parts = []
                if 6 in PARTS[i]:
                    parts.append((0, 64, wall[:, WOFF[6]:WOFF[6] + 64], None,
                                  lambda cs: hm5s[i][:, 0, cs]))
                if 7 in PARTS[i]:
                    parts.append((64, 96, w789[0:64, 0:32], (0, 64),
                                  lambda cs: prev["hr678"][0:64, 0, cs]))
                if 8 in PARTS[i]:
                    parts.append((96, 112, w789[64:96, 32:48], (64, 96),
                                  lambda cs: prev["hr678"][64:96, 0, cs]))
                sl["p_lo"] = parts[0][0]
                sl["p_hi"] = parts[-1][1]
                # weight-major: one LDW per weight per slot (consecutive dedup),
                # streams of different col-groups overlap on the PE
                for p0, p1, wap, tpos, rhs_of in parts:
                    for u in wins:
                        for t in range(nmm):
                            cs = slice(u * wsz + t * MSUB, u * wsz + (t + 1) * MSUB)
                            out_sl = zhs[u][p0:p1, bass.ts(t, MSUB)]
                            if tpos is None:
                                nc.tensor.matmul(out_sl, wap, rhs_of(cs),
                                                 start=True, stop=True)
                            else:
                                nc.tensor.matmul(out_sl, wap, rhs_of(cs),
                                                 start=True, stop=True,
                                                 tile_position=tpos)
                if i in hm5s:
                    del hm5s[i]

            def emit_slot_drainmask(i):
                sl = slots[i]
                p_lo, p_hi = sl["p_lo"], sl["p_hi"]
                hr678 = sl["hr678"]
                m678 = m678s[i]
                wins, wsz = slot_wins(sl["fine"])
                for u in wins:
                    dst = hr678[p_lo:p_hi, 0, u * wsz:(u + 1) * wsz]
                    drain_relu("dve" if u % 2 == 0 else "act", dst,
                               sl["zhs"][u][p_lo:p_hi, :], b678[p_lo:p_hi, 0:1])
                    # hs1 half -> gps only at coarse granularity (slack: consumers
                    # are a block away except L9, which is ~4us later)
                    mask_mul("gps" if (not sl["fine"] and u == 1 and i < 3)
                             else "dve",
                             dst, dst, m678[p_lo:p_hi, 0, u * wsz:(u + 1) * wsz])
                del sl["zhs"]

            def emit_slot_l9(i):
                b = i - 2
                sl = slots[i]
                hm678 = sl["hr678"]
                fine = sl["fine"]
                osb = outp.tile([10, BLK], f32, tag="osb", bufs=2, name=f"osb_{b}")
                wins, wsz = slot_wins(fine)
                nmm = wsz // MSUB
                for u in wins:
                    z9 = psp.tile([128, wsz], f32, tag="ps", name=f"z9_{b}_{u}")
                    for t in range(nmm):
                        rhs_sl = slice(u * wsz + t * MSUB, u * wsz + (t + 1) * MSUB)
                        nc.tensor.matmul(z9[0:10, bass.ts(t, MSUB)],
                                         w789[96:112, 48:58],
                                         hm678[96:112, 0, rhs_sl],
                                         start=True, stop=True, tile_position=(96, 0))
                    if u % 2 == 0:
                        nc.scalar.activation(osb[:, u * wsz:(u + 1) * wsz],
                                             z9[0:10, :], IDENT, bias=b9[:, 0:1])
                    else:
                        nc.vector.tensor_scalar(osb[:, u * wsz:(u + 1) * wsz],
                                                z9[0:10, :], b9[:, 0:1], None, ADD)
                    if (u + 1) * wsz == DSUB or (u + 1) * wsz == BLK:
                        h0 = (u + 1) * wsz - DSUB
                        nc.sync.dma_start(
                            out_d[:, b * BLK + h0:b * BLK + h0 + DSUB],
                            osb[:, h0:h0 + DSUB])
                if (i - 1) in slots:
                    del slots[i - 1]

            # --- schedule ---------------------------------------------------------
            # PE warmup: dummy matmuls on a memset scratch tile so the HAM clock
            # gate opens before real work arrives (weights/x still in DMA).
            nc.vector.memset(scratch[:], 0.0)
            zw = psp.tile([128, MSUB], f32, tag="ps", name="zwarm")
            for i in range(13):
                nc.tensor.matmul(zw[:], scratch[:, 0:128], scratch[:], start=True,
                                 stop=True)

            def warm(n):
                # filler matmuls that keep the PE HAM activity window busy while
                # a dependency chain stalls the real stream (tail)
                zf = psp.tile([128, MSUB], f32, tag="ps", name="zf")
                for _ in range(n):
                    nc.tensor.matmul(zf[:], scratch[:, 0:128], scratch[:],
                                     start=True, stop=True)

            issue_pack_dmas(0)
            emit_layer(0, 0)                       # L1(0) during startup
            for b in range(NBLK):
                if b + 1 < NBLK:
                    issue_pack_dmas(b + 1)
                if b == 2:
                    issue_m678_dma(4)              # phantom tail-slot planes
                if b == 3:
                    issue_m678_dma(5)
                if b >= 1:
                    warm(3)                        # bridge the block handover
                emit_layer(b, 1)                   # L2
                if b >= 1:
                    emit_slot_trio(b - 1)          # L6(b-1)+L7(b-2)+L8(b-3)
                emit_layer(b, 2)                   # L3
                if b >= 1:
                    emit_slot_drainmask(b - 1)
                emit_layer(b, 3)                   # L4
                if b == NBLK - 1:
                    emit_slot_trio(3)              # L7(2)+L8(1): no block-3 deps
                    emit_slot_drainmask(3)
                if b + 1 < NBLK:
                    emit_layer(b + 1, 0)           # L1(b+1) pipelined ahead
                else:
                    warm(5)                        # fill the L1-ahead hole
                if b >= 1 and b - 1 >= 2:
                    emit_slot_l9(b - 1)            # out block 0
                emit_layer(b, 4)                   # L5
                hm5s[b] = hrs.pop((b, 4))

            # tail: only block 3's own ladder (v3-style, fine windows) remains on
            # the critical chain; the independent slot-4/l9 work and warm()
            # fillers run during each ladder step's drain+mask latency.
            lad = hrp.tile([128, 1, BLK], f16, tag="hr678", name="lad3", bufs=2)
            m6d = m678s[5]                         # diagonal plane {m6,m7,m8}(3)
            LADW = [(0, 64, wall[:, WOFF[6]:WOFF[6] + 64], None, None),
                    (64, 96, w789[0:64, 0:32], (0, 64), (0, 64)),
                    (96, 112, w789[64:96, 32:48], (64, 96), (64, 96))]

            def lad_step(step):
                p0, p1, wap, tpos, brg = LADW[step]
                for w in range(NSUB):
                    zh = psp.tile([128, MSUB], f32, tag="ps", name=f"lz_{step}_{w}")
                    cs = slice(w * MSUB, (w + 1) * MSUB)
                    rhs = (hm5s[3][:, 0, cs] if step == 0 else
                           lad[brg[0]:brg[1], 0, cs])
                    if tpos is None:
                        nc.tensor.matmul(zh[p0:p1, :], wap, rhs, start=True,
                                         stop=True)
                    else:
                        nc.tensor.matmul(zh[p0:p1, :], wap, rhs, start=True,
                                         stop=True, tile_position=tpos)
                    dst = lad[p0:p1, 0, cs]
                    drain_relu("dve" if w % 2 == 0 else "act", dst, zh[p0:p1, :],
                               b678[p0:p1, 0:1])
                    mask_mul("dve", dst, dst, m6d[p0:p1, 0, cs])

            emit_slot_l9(3)                        # out block 1 (fills L5 drain wait)
            lad_step(0)                            # L6(3)
            emit_slot_trio(4)                      # L8(2): independent, fills wait
            emit_slot_drainmask(4)
            lad_step(1)                            # L7(3)
            emit_slot_l9(4)                        # out block 2: fills wait
            lad_step(2)                            # L8(3)
            warm(5)
            slots[5] = {"hr678": lad, "fine": True, "p_lo": 0, "p_hi": 112}
            emit_slot_l9(5)                        # out block 3

    _dedup_ldweights(nc)
    nc.compile()
    return nc


def _get_program():
    if "nc" not in _PROG:
        _PROG["nc"] = _build_program()
    return _PROG["nc"]


def _host_prep(inputs):
    """Build per-core input maps (numpy only)."""
    x = np.asarray(inputs["x"], dtype=np.float32)
    Ws = [np.asarray(inputs[f"W{i}"], dtype=np.float32) for i in range(1, 10)]
    bs = [np.asarray(inputs[f"b{i}"], dtype=np.float32) for i in range(1, 10)]
    ms = [np.asarray(inputs[f"m{i}"], dtype=np.float32) for i in range(1, 9)]

    # fold dropout scale into next layer's weights; binarize masks
    Wf = [Ws[0]]
    for i in range(1, 9):
        s = float(ms[i - 1].max())
        if s <= 0.0:  # degenerate all-dropped mask; keep weights unscaled
            s = 1.0
        Wf.append(Ws[i] * np.float32(s))

    # weight blob: w1@0 w2@256 w3@512 w4@1536 w5@2560 w6@2816 w789@2880
    WOFF = {1: 0, 2: 256, 3: 512, 4: 1536, 5: 2560, 6: 2816, 789: 2880}
    wb = np.zeros((128, 2944), dtype=np.float16)
    for l in range(1, 7):
        W = Wf[l - 1]
        K, N = W.shape
        for k in range((K + 127) // 128):
            blk = W[k * 128:(k + 1) * 128].astype(np.float16)
            wb[: blk.shape[0], WOFF[l] + k * N: WOFF[l] + k * N + N] = blk
    wb[0:64, 2880:2912] = Wf[6].astype(np.float16)    # W7
    wb[64:96, 2912:2928] = Wf[7].astype(np.float16)   # W8
    wb[96:112, 2928:2938] = Wf[8].astype(np.float16)  # W9
    wb1, wb = np.ascontiguousarray(wb[:, 0:256]), np.ascontiguousarray(wb[:, 256:])
    bb = np.zeros((128, 12), dtype=np.float32)
    bb[:, 0] = bs[0]
    bb[:, 1], bb[:, 2] = bs[1][0:128], bs[1][128:256]
    for c in range(4):
        bb[:, 3 + c] = bs[2][c * 128:(c + 1) * 128]
    bb[:, 7], bb[:, 8] = bs[3][0:128], bs[3][128:256]
    bb[:, 9] = bs[4]
    bb[0:64, 10], bb[64:96, 10], bb[96:112, 10] = bs[5], bs[6], bs[7]
    bb[0:10, 11] = bs[8]
    shared = {"WB1": wb1, "WB": wb, "BB": bb}

    in_maps = []
    for c in range(NCORES):
        sl = slice(c * SHARD, (c + 1) * SHARD)
        pack = np.zeros((NBLK, 128, NPACK, BLK), dtype=np.float16)
        m678 = np.zeros((NSLOT, 128, 1, BLK), dtype=np.float16)
        xT = x[sl].T  # (256, SHARD)
        mT = [None] + [(ms[i][sl] != 0).T.astype(np.float16) for i in range(8)]
        for b in range(NBLK):
            cs = slice(b * BLK, (b + 1) * BLK)
            pack[b, :, 0, :] = xT[0:128, cs]
            pack[b, :, 1, :] = xT[128:256, cs]
            pack[b, :, 2, :] = mT[1][:, cs]
            pack[b, :, 3, :], pack[b, :, 4, :] = mT[2][0:128, cs], mT[2][128:256, cs]
            for k in range(4):
                pack[b, :, 5 + k, :] = mT[3][k * 128:(k + 1) * 128, cs]
            pack[b, :, 9, :], pack[b, :, 10, :] = mT[4][0:128, cs], mT[4][128:256, cs]
            pack[b, :, 11, :] = mT[5][:, cs]
        # slot-staggered m678 planes: slot i masks {m6(i), m7(i-1), m8(i-2)};
        # plane 5 is block 3's diagonal {m6(3), m7(3), m8(3)} for the tail ladder
        for i in range(NSLOT - 1):
            if i < NBLK:
                m678[i, 0:64, 0, :] = mT[6][:, i * BLK:(i + 1) * BLK]
            if 0 <= i - 1 < NBLK:
                m678[i, 64:96, 0, :] = mT[7][:, (i - 1) * BLK:i * BLK]
            if 0 <= i - 2 < NBLK:
                m678[i, 96:112, 0, :] = mT[8][:, (i - 2) * BLK:(i - 1) * BLK]
        lb = slice((NBLK - 1) * BLK, NBLK * BLK)
        m678[5, 0:64, 0, :] = mT[6][:, lb]
        m678[5, 64:96, 0, :] = mT[7][:, lb]
        m678[5, 96:112, 0, :] = mT[8][:, lb]
        in_maps.append({"pack": pack, "M678": m678, **shared})
    return in_maps


def kernel(**inputs) -> np.ndarray:
    from concourse.bass_utils import run_bass_kernel_spmd

    nc = _get_program()
    in_maps = _host_prep(inputs)
    res = run_bass_kernel_spmd(nc, in_maps, list(range(NCORES)))
    out = np.empty((BATCH, DIMS[-1]), dtype=np.float32)
    for c in range(NCORES):
        out[c * SHARD:(c + 1) * SHARD, :] = res.results[c]["outT"].T
    return out


# revision 25
# speedup vs baseline: 1.2843x; 1.0300x over previous
"""Trainium2 Bass kernel for the 9-layer dense MLP (dropout-mask training forward).

Strategy (pure data parallel, 8 cores, 8192 batch rows each):
  - Activations kept transposed on-chip: features on partitions, batch cols on free dim.
    Each layer computes zT = W^T @ hT via nc.tensor.matmul(out, lhsT=W, rhs=hT).
  - fp16 weights/activations/masks (fp32 PSUM accumulation), fp32 biases + output.
  - Dropout masks binarized on host ({0,1} fp16); the 1/keep scale is folded into the
    next layer's weights.
  - Host pack layout [NBLK, 128, NPACK, BLK] so each per-block DMA is contiguous per
    partition (2 DMAs per block: x+m1, then the remaining masks).
  - PSUM: one shared pool of 4x [128,1024] fp32 tiles (8 banks). Matmuls write 512-col
    halves; drains are single FD=1024 instructions (fused bias+relu) split ~3:1
    ACT:DVE; mask multiplies are FD=2048 DVE tensor_tensor with a measured dose
    offloaded to GpSimd.
  - Small layers 6/7/8 partition-packed (offsets 0/64/96 via matmul tile_position);
    each ladder step drains immediately to SBUF so its PSUM tile recycles fast.
    Block b's ladder is software-pipelined into block b+1's big-layer bursts.
  - A short burst of dummy matmuls at t~1us keeps the PE HAM activity monitor busy so
    the array is at full clock (K=8/8) when real work arrives.
"""

import sys

sys.path.insert(0, "/opt/trn_rl_repo")

import numpy as np

DIMS = [256, 128, 256, 512, 256, 128, 64, 32, 16, 10]
NCORES = 8
BATCH = 65536
SHARD = BATCH // NCORES  # 8192
MSUB = 512               # matmul N (PSUM bank limit for fp32)
DSUB = 1024              # drain granularity (2 banks)
BLK = 2048               # block columns
NBLK = SHARD // BLK      # 4
NSUB = BLK // MSUB       # 4
NDR = BLK // DSUB        # 2

# pack chunk layout (each chunk = 128 partitions x BLK cols, fp16), per block:
#   0,1: xT   2: m1   3,4: m2   5-8: m3   9,10: m4   11: m5
# m678 ships separately as NBLK+2 slot-staggered planes (rows 0:64 = m6(slot),
# 64:96 = m7(slot-1), 96:112 = m8(slot-2)) to match the slot-fused ladder.
NPACK = 12
NSLOT = NBLK + 2

_PROG = {}


def _raise_sbuf_cap():
    # tile_utils.max_sbuf_usage is a stale 192KB constant; cayman has 208KB usable.
    import concourse.tile_utils as tu

    if getattr(tu, "max_sbuf_usage", 0) < 206 * 1024:
        tu.max_sbuf_usage = 206 * 1024


def _dedup_ldweights(nc):
    """Remove back-to-back redundant LDWEIGHTS (same stationary operand) so
    consecutive same-weight matmuls pipeline on the PE. Only drops LDW
    instructions that carry no semaphore waits/updates."""
    removed = 0
    for fn in nc.m.functions:
        for blk in fn.blocks:
            il = blk.instructions
            keep, last_sig = [], None
            for inst in il:
                nm = type(inst).__name__
                if nm == "InstLdweights":
                    sig = (str(inst.ins[0]), str(inst.is_transpose), str(inst.perf_mode),
                           str(getattr(inst, "tile_position", None)))
                    si = inst.sync_info
                    clean = si is None or (not si.on_wait and not si.on_update)
                    if sig == last_sig and clean:
                        removed += 1
                        continue
                    last_sig = sig
                keep.append(inst)
            if removed and len(keep) != len(il):
                while il:
                    il.pop()
                il.extend(keep)
    return removed


def _build_program():
    import concourse.bass as bass
    import concourse.tile as tile
    from concourse import bacc, mybir

    _raise_sbuf_cap()

    f16 = mybir.dt.float16
    f32 = mybir.dt.float32
    RELU = mybir.ActivationFunctionType.Relu
    IDENT = mybir.ActivationFunctionType.Identity
    ADD = mybir.AluOpType.add
    MAX = mybir.AluOpType.max

    nc = bacc.Bacc("TRN2", target_bir_lowering=False, debug=False, num_devices=NCORES)

    pack_d = nc.dram_tensor("pack", [NBLK, 128, NPACK, BLK], f16, kind="ExternalInput").ap()
    m678_d = nc.dram_tensor("M678", [NSLOT, 128, 1, BLK], f16, kind="ExternalInput").ap()
    # weights in two host-laid-out fp16 blobs (W1 separate so the first
    # LDWEIGHTS isn't gated on the full blob), biases in one fp32 blob
    wb1_d = nc.dram_tensor("WB1", [128, 256], f16, kind="ExternalInput").ap()
    wb_d = nc.dram_tensor("WB", [128, 2688], f16, kind="ExternalInput").ap()
    bb_d = nc.dram_tensor("BB", [128, 12], f32, kind="ExternalInput").ap()
    out_d = nc.dram_tensor("outT", [10, SHARD], f32, kind="ExternalOutput").ap()

    with tile.TileContext(nc) as tc:
        with (
            tc.tile_pool(name="wpool", bufs=1) as wp,
            tc.tile_pool(name="mk", bufs=2) as mkp,
            tc.tile_pool(name="hr", bufs=1) as hrp,
            tc.tile_pool(name="osb", bufs=2) as outp,
            tc.tile_pool(name="ps", bufs=4, space="PSUM") as psp,
        ):
            wall = wp.tile([128, 2944], f16, tag="wall")
            ball = wp.tile([128, 12], f32, tag="ball")
            scratch = wp.tile([128, 512], f16, tag="scratch")
            # blob column offsets: w1@0(256) w2@256(256) w3@512(1024) w4@1536(1024)
            #   w5@2560(256) w6@2816(64) w789@2880(64: W7 r0-63 c0-31, W8 r64-95
            #   c32-47, W9 r96-111 c48-57)
            WOFF = {1: 0, 2: 256, 3: 512, 4: 1536, 5: 2560, 6: 2816, 789: 2880}
            w789 = wall[:, WOFF[789]:WOFF[789] + 64]
            b15 = ball[:, 0:10]
            b678 = ball[:, 10:11]
            b9 = ball[0:10, 11:12]

            def wslice(l, k, c, N):
                base = WOFF[l] + k * N
                return wall[:, base + c * 128: base + (c + 1) * 128]

            def drain_relu(eng, dst, zsrc, bias_ap):
                if eng == "act":
                    nc.scalar.activation(dst, zsrc, RELU, bias=bias_ap)
                else:
                    nc.vector.tensor_scalar(dst, zsrc, bias_ap, 0.0, ADD, MAX)

            def mask_mul(eng, dst, src, msrc):
                if eng == "gps":
                    nc.gpsimd.tensor_mul(dst, src, msrc)
                else:
                    nc.vector.tensor_mul(dst, src, msrc)

            # drain engine picker: ~70:30 act:dve (ACT is cheaper per element but
            # DVE has mask work too; this balances their queues)
            dr_i = [0]

            def pick_drain():
                i = dr_i[0]
                dr_i[0] += 1
                return "dve" if i % 10 in (2, 5, 8) else "act"

            state = {}
            packs = {}
            hrs = {}

            def issue_pack_dmas(b):
                # per-chunk tiles/DMAs: each mask tile's ring slot is released as
                # soon as its own layer consumes it, so block b+2's DMAs start
                # early instead of waiting for ALL of block b's masks (WAR).
                pk3 = mkp.tile([128, 3, BLK], f16, tag="pk3", name=f"pk3_{b}")
                m2t = mkp.tile([128, 2, BLK], f16, tag="m2", name=f"m2_{b}")
                m3t = mkp.tile([128, 4, BLK], f16, tag="m3", name=f"m3_{b}")
                m4t = mkp.tile([128, 2, BLK], f16, tag="m4", name=f"m4_{b}")
                m5t = mkp.tile([128, 1, BLK], f16, tag="m5", name=f"m5_{b}")
                if b == 0:
                    nc.sync.dma_start(wall[:, 0:256], wb1_d[:])
                    nc.sync.dma_start(ball[:], bb_d[:])
                    for q in range(NSUB):
                        qs = bass.ts(q, MSUB)
                        nc.sync.dma_start(pk3[:, :, qs], pack_d[0, :, 0:3, qs])
                    nc.sync.dma_start(wall[:, 256:], wb_d[:])
                else:
                    nc.sync.dma_start(pk3[:], pack_d[b, :, 0:3, :])
                nc.sync.dma_start(m2t[:], pack_d[b, :, 3:5, :])
                nc.sync.dma_start(m3t[:], pack_d[b, :, 5:9, :])
                nc.sync.dma_start(m4t[:], pack_d[b, :, 9:11, :])
                nc.sync.dma_start(m5t[:], pack_d[b, :, 11:12, :])
                packs[b] = (pk3, m2t, m3t, m4t, m5t)
                issue_m678_dma(b)

            m678s = {}

            def issue_m678_dma(i):
                m678s[i] = mkp.tile([128, 1, BLK], f16, tag="m678", name=f"m678s_{i}",
                                    bufs=3)
                nc.sync.dma_start(m678s[i][:], m678_d[i])

            # (Kc, layer, wN, Cc, bias_off, hrtag)
            LAYER_CFG = [
                (2, 1, 128, 1, 0, "hr1"),
                (1, 2, 256, 2, 1, "hr2"),
                (2, 3, 512, 4, 3, "hr3"),
                (4, 4, 256, 2, 7, "hr4"),
                (2, 5, 128, 1, 9, "hr5"),
            ]
            # mask engine schedule per layer index. GpSimd is ~4x slower than DVE
            # per element, so it only gets masks with slack before their consumer:
            # m1 (L1 runs a block ahead) and m5 (ladder consumes it next block).
            # L2/L3/L4 masks sit on the next layer's critical path -> DVE only.
            MASK_MODE = {
                0: ["dve2"],
                1: ["dve2", "dve2"],
                2: ["dve2", "dve2", "dve2", "dve2"],
                3: ["dve2", "dve2"],
                4: ["dve2"],
            }
            # per-layer drain engine assignment, in (u, c) order. ACT-heavy for
            # the mid-block bulk (L3/L4); the boundary-critical L2 keeps one on
            # DVE so its chain isn't stuck behind the ACT queue.
            DRAIN_ENG = {
                0: ["dve", "act"],
                1: ["act", "act", "dve", "act"],
                2: ["act", "act", "act", "act", "act", "act", "act", "act"],
                3: ["act", "act", "act", "act"],
                4: ["act", "act"],
            }

            def emit_layer(b, li):
                Kc, wl, wN, Cc, boff, hrtag = LAYER_CFG[li]
                hr = hrp.tile([128, Cc, BLK], f16, tag=hrtag, name=hrtag + f"_{b}",
                              bufs=2 if hrtag in ("hr5", "hr2", "hr1") else 1)
                pk3 = packs[b][0]
                hin = pk3 if li == 0 else hrs[(b, li - 1)]

                def msl(c, cols):
                    if li == 0:
                        return pk3[:, 2, cols]
                    return packs[b][li][:, c, cols]

                zs = {}
                for c in range(Cc):
                    for u in range(NDR):
                        zs[c, u] = psp.tile([128, DSUB], f32, tag="ps",
                                            name=f"z_{hrtag}_{b}_{c}_{u}")
                if b == 0 and li == 0:
                    # startup: 512-col windows so the first matmul fires as soon
                    # as the first x quarter lands
                    for t in range(NSUB):
                        ts_ = bass.ts(t, MSUB)
                        for k in range(Kc):
                            nc.tensor.matmul(
                                zs[0, t // 2][:, bass.ts(t % 2, MSUB)],
                                wslice(wl, k, 0, wN), hin[:, k, ts_],
                                start=(k == 0), stop=(k == Kc - 1))
                        drain_relu("dve" if t % 2 == 0 else "act",
                                   hr[:, 0, ts_], zs[0, t // 2][:, bass.ts(t % 2, MSUB)],
                                   b15[:, 0:1])
                        mask_mul("dve", hr[:, 0, ts_], hr[:, 0, ts_], msl(0, ts_))
                    hrs[(b, li)] = hr
                    return
                # weight-major matmuls so consecutive MMs share one LDWEIGHTS
                for c in range(Cc):
                    for k in range(Kc):
                        wap = wslice(wl, k, c, wN)
                        for t in range(NSUB):
                            nc.tensor.matmul(
                                zs[c, t // 2][:, bass.ts(t % 2, MSUB)], wap,
                                hin[:, k, bass.ts(t, MSUB)],
                                start=(k == 0), stop=(k == Kc - 1))
                de = DRAIN_ENG[li]
                for u in range(NDR):
                    for c in range(Cc):
                        drain_relu(de[u * Cc + c], hr[:, c, bass.ts(u, DSUB)],
                                   zs[c, u][:], b15[:, boff + c:boff + c + 1])
                full = slice(0, BLK)
                for c in range(Cc):
                    mode = MASK_MODE[li][c]
                    if mode == "dve":
                        mask_mul("dve", hr[:, c, full], hr[:, c, full], msl(c, full))
                    elif mode == "dve2":
                        for uu in range(NDR):
                            hs = bass.ts(uu, DSUB)
                            mask_mul("dve", hr[:, c, hs], hr[:, c, hs], msl(c, hs))
                    else:
                        hs0, hs1 = bass.ts(0, DSUB), bass.ts(1, DSUB)
                        mask_mul("dve", hr[:, c, hs0], hr[:, c, hs0], msl(c, hs0))
                        mask_mul("gps", hr[:, c, hs1], hr[:, c, hs1], msl(c, hs1))
                hrs[(b, li)] = hr
                if li > 0:
                    del hrs[(b, li - 1)]

            # --- small-layer ladder, slot-fused across blocks --------------------
            # Slot i co-issues three INDEPENDENT small matmuls from staggered
            # blocks into disjoint partition ranges of ONE PSUM tile:
            #   rows 0:64   L6(block i)    cols 0:64   of the PE array
            #   rows 64:96  L7(block i-1)  cols 64:96  (tile_position (0,64))
            #   rows 96:112 L8(block i-2)  cols 96:128 (tile_position (64,96))
            # They run concurrently (disjoint subarrays), and ONE drain + ONE
            # mask serves all three (the host staggers the m678 mask planes the
            # same way). L9(block i-2) then reads rows 96:112 after the mask.
            slots = {}
            hm5s = {}

            def slot_wins(fine):
                return (range(NSUB), MSUB) if fine else (range(NDR), DSUB)

            def emit_slot_trio(i, fine=False):
                sl = slots.setdefault(i, {})
                sl["hr678"] = hrp.tile([128, 1, BLK], f16, tag="hr678",
                                       name=f"hr678_{i}", bufs=2)
                prev = slots.get(i - 1)
                wins, wsz = slot_wins(fine)
                nmm = wsz // MSUB
                zhs = {}
                for u in wins:
                    zhs[u] = psp.tile([128, wsz], f32, tag="ps", name=f"zh_{i}_{u}")
                sl["zhs"] = zhs
                sl["fine"] = fine
                # which small layers live in slot i: slots 0..2 accumulate the
                # stagger; slots 3 (L7+L8) and 4 (L8) run inside block 3; block
                # 3's own ladder runs v3-style in the tail.
                PARTS = {0: (6,), 1: (6, 7), 2: (6, 7, 8), 3: (7, 8), 4: (8,)}
                parts = []
                if 6 in PARTS[i]:
                    parts.append((0, 64, wall[:, WOFF[6]:WOFF[6] + 64], None,
                                  lambda cs: hm5s[i][:, 0, cs]))
                if 7 in PARTS[i]:
                    parts.append((64, 96, w789[0:64, 0:32], (0, 64),
                                  lambda cs: prev["hr678"][0:64, 0, cs]))
                if 8 in PARTS[i]:
                    parts.append((96, 112, w789[64:96, 32:48], (64, 96),
                                  lambda cs: prev["hr678"][64:96, 0, cs]))
                sl["p_lo"] = parts[0][0]
                sl["p_hi"] = parts[-1][1]
                # weight-major: one LDW per weight per slot (consecutive dedup),
                # streams of different col-groups overlap on the PE
                for p0, p1, wap, tpos, rhs_of in parts:
                    for u in wins:
                        for t in range(nmm):
                            cs = slice(u * wsz + t * MSUB, u * wsz + (t + 1) * MSUB)
                            out_sl = zhs[u][p0:p1, bass.ts(t, MSUB)]
                            if tpos is None:
                                nc.tensor.matmul(out_sl, wap, rhs_of(cs),
                                                 start=True, stop=True)
                            else:
                                nc.tensor.matmul(out_sl, wap, rhs_of(cs),
                                                 start=True, stop=True,
                                                 tile_position=tpos)
                if i in hm5s:
                    del hm5s[i]

            def emit_slot_drainmask(i):
                sl = slots[i]
                p_lo, p_hi = sl["p_lo"], sl["p_hi"]
                hr678 = sl["hr678"]
                m678 = m678s[i]
                wins, wsz = slot_wins(sl["fine"])
                for u in wins:
                    dst = hr678[p_lo:p_hi, 0, u * wsz:(u + 1) * wsz]
                    drain_relu("dve" if u % 2 == 0 else "act", dst,
                               sl["zhs"][u][p_lo:p_hi, :], b678[p_lo:p_hi, 0:1])
                    # hs1 half -> gps only at coarse granularity (slack: consumers
                    # are a block away except L9, which is ~4us later)
                    mask_mul("gps" if (not sl["fine"] and u == 1 and i < 3)
                             else "dve",
                             dst, dst, m678[p_lo:p_hi, 0, u * wsz:(u + 1) * wsz])
                del sl["zhs"]

            def emit_slot_l9(i):
                b = i - 2
                sl = slots[i]
                hm678 = sl["hr678"]
                fine = sl["fine"]
                osb = outp.tile([10, BLK], f32, tag="osb", bufs=2, name=f"osb_{b}")
                wins, wsz = slot_wins(fine)
                nmm = wsz // MSUB
                for u in wins:
                    z9 = psp.tile([128, wsz], f32, tag="ps", name=f"z9_{b}_{u}")
                    for t in range(nmm):
                        rhs_sl = slice(u * wsz + t * MSUB, u * wsz + (t + 1) * MSUB)
                        nc.tensor.matmul(z9[0:10, bass.ts(t, MSUB)],
                                         w789[96:112, 48:58],
                                         hm678[96:112, 0, rhs_sl],
                                         start=True, stop=True, tile_position=(96, 0))
                    if u % 2 == 0:
                        nc.scalar.activation(osb[:, u * wsz:(u + 1) * wsz],
                                             z9[0:10, :], IDENT, bias=b9[:, 0:1])
                    else:
                        nc.vector.tensor_scalar(osb[:, u * wsz:(u + 1) * wsz],
                                                z9[0:10, :], b9[:, 0:1], None, ADD)
                    if (u + 1) * wsz == DSUB or (u + 1) * wsz == BLK:
                        h0 = (u + 1) * wsz - DSUB
                        nc.sync.dma_start(
                            out_d[:, b * BLK + h0:b * BLK + h0 + DSUB],
                            osb[:, h0:h0 + DSUB])
                if (i - 1) in slots:
                    del slots[i - 1]

            # --- schedule ---------------------------------------------------------
            # PE warmup: dummy matmuls on a memset scratch tile so the HAM clock
            # gate opens before real work arrives (weights/x still in DMA).
            nc.vector.memset(scratch[:], 0.0)
            zw = psp.tile([128, MSUB], f32, tag="ps", name="zwarm")
            for i in range(13):
                nc.tensor.matmul(zw[:], scratch[:, 0:128], scratch[:], start=True,
                                 stop=True)

            def warm(n):
                # filler matmuls that keep the PE HAM activity window busy while
                # a dependency chain stalls the real stream (tail)
                zf = psp.tile([128, MSUB], f32, tag="ps", name="zf")
                for _ in range(n):
                    nc.tensor.matmul(zf[:], scratch[:, 0:128], scratch[:],
                                     start=True, stop=True)

            issue_pack_dmas(0)
            emit_layer(0, 0)                       # L1(0) during startup
            for b in range(NBLK):
                if b + 1 < NBLK:
                    issue_pack_dmas(b + 1)
                if b == 2:
                    issue_m678_dma(4)              # phantom tail-slot planes
                if b == 3:
                    issue_m678_dma(5)
                if b >= 1:
                    warm(3)                        # bridge the block handover
                emit_layer(b, 1)                   # L2
                if b >= 1:
                    emit_slot_trio(b - 1)          # L6(b-1)+L7(b-2)+L8(b-3)
                emit_layer(b, 2)                   # L3
                if b >= 1:
                    emit_slot_drainmask(b - 1)
                emit_layer(b, 3)                   # L4
                if b == NBLK - 1:
                    emit_slot_trio(3)              # L7(2)+L8(1): no block-3 deps
                    emit_slot_drainmask(3)
                if b + 1 < NBLK:
                    emit_layer(b + 1, 0)           # L1(b+1) pipelined ahead
                else:
                    warm(5)                        # fill the L1-ahead hole
                if b >= 1 and b - 1 >= 2:
                    emit_slot_l9(b - 1)            # out block 0
                emit_layer(b, 4)                   # L5
                hm5s[b] = hrs.pop((b, 4))

            # tail: only block 3's own ladder (v3-style, fine windows) remains on
            # the critical chain; the independent slot-4/l9 work and warm()
            # fillers run during each ladder step's drain+mask latency.
            lad = hrp.tile([128, 1, BLK], f16, tag="hr678", name="lad3", bufs=2)
            m6d = m678s[5]                         # diagonal plane {m6,m7,m8}(3)
            LADW = [(0, 64, wall[:, WOFF[6]:WOFF[6] + 64], None, None),
                    (64, 96, w789[0:64, 0:32], (0, 64), (0, 64)),
                    (96, 112, w789[64:96, 32:48], (64, 96), (64, 96))]

            def lad_step(step):
                p0, p1, wap, tpos, brg = LADW[step]
                for w in range(NSUB):
                    zh = psp.tile([128, MSUB], f32, tag="ps", name=f"lz_{step}_{w}")
                    cs = slice(w * MSUB, (w + 1) * MSUB)
                    rhs = (hm5s[3][:, 0, cs] if step == 0 else
                           lad[brg[0]:brg[1], 0, cs])
                    if tpos is None:
                        nc.tensor.matmul(zh[p0:p1, :], wap, rhs, start=True,
                                         stop=True)
                    else:
                        nc.tensor.matmul(zh[p0:p1, :], wap, rhs, start=True,
                                         stop=True, tile_position=tpos)
                    dst = lad[p0:p1, 0, cs]
                    drain_relu("dve" if w % 2 == 0 else "act", dst, zh[p0:p1, :],
                               b678[p0:p1, 0:1])
                    mask_mul("dve", dst, dst, m6d[p0:p1, 0, cs])

            warm(4)
            emit_slot_l9(3)                        # out block 1 (fills L5 drain wait)
            lad_step(0)                            # L6(3)
            emit_slot_trio(4)                      # L8(2): independent, fills wait
            emit_slot_drainmask(4)
            lad_step(1)                            # L7(3)
            emit_slot_l9(4)                        # out block 2: fills wait
            lad_step(2)                            # L8(3)
            warm(5)
            slots[5] = {"hr678": lad, "fine": True, "p_lo": 0, "p_hi": 112}
            emit_slot_l9(5)                        # out block 3

    _dedup_ldweights(nc)
    nc.compile()
    return nc


def _get_program():
    if "nc" not in _PROG:
        _PROG["nc"] = _build_program()
    return _PROG["nc"]


def _host_prep(inputs):
    """Build per-core input maps (numpy only)."""
    x = np.asarray(inputs["x"], dtype=np.float32)
    Ws = [np.asarray(inputs[f"W{i}"], dtype=np.float32) for i in range(1, 10)]
    bs = [np.asarray(inputs[f"b{i}"], dtype=np.float32) for i in range(1, 10)]
    ms = [np.asarray(inputs[f"m{i}"], dtype=np.float32) for i in range(1, 9)]

    # fold dropout scale into next layer's weights; binarize masks
    Wf = [Ws[0]]
    for i in range(1, 9):
        s = float(ms[i - 1].max())
        if s <= 0.0:  # degenerate all-dropped mask; keep weights unscaled
            s = 1.0
        Wf.append(Ws[i] * np.float32(s))

    # weight blob: w1@0 w2@256 w3@512 w4@1536 w5@2560 w6@2816 w789@2880
    WOFF = {1: 0, 2: 256, 3: 512, 4: 1536, 5: 2560, 6: 2816, 789: 2880}
    wb = np.zeros((128, 2944), dtype=np.float16)
    for l in range(1, 7):
        W = Wf[l - 1]
        K, N = W.shape
        for k in range((K + 127) // 128):
            blk = W[k * 128:(k + 1) * 128].astype(np.float16)
            wb[: blk.shape[0], WOFF[l] + k * N: WOFF[l] + k * N + N] = blk
    wb[0:64, 2880:2912] = Wf[6].astype(np.float16)    # W7
    wb[64:96, 2912:2928] = Wf[7].astype(np.float16)   # W8
    wb[96:112, 2928:2938] = Wf[8].astype(np.float16)  # W9
    wb1, wb = np.ascontiguousarray(wb[:, 0:256]), np.ascontiguousarray(wb[:, 256:])
    bb = np.zeros((128, 12), dtype=np.float32)
    bb[:, 0] = bs[0]
    bb[:, 1], bb[:, 2] = bs[1][0:128], bs[1][128:256]
    for c in range(4):
        bb[:, 3 + c] = bs[2][c * 128:(c + 1) * 128]
    bb[:, 7], bb[:, 8] = bs[3][0:128], bs[3][128:256]
    bb[:, 9] = bs[4]
    bb[0:64, 10], bb[64:96, 10], bb[96:112, 10] = bs[5], bs[6], bs[7]
    bb[0:10, 11] = bs[8]
    shared = {"WB1": wb1, "WB": wb, "BB": bb}

    in_maps = []
    for c in range(NCORES):
        sl = slice(c * SHARD, (c + 1) * SHARD)
        pack = np.zeros((NBLK, 128, NPACK, BLK), dtype=np.float16)
        m678 = np.zeros((NSLOT, 128, 1, BLK), dtype=np.float16)
        xT = x[sl].T  # (256, SHARD)
        mT = [None] + [(ms[i][sl] != 0).T.astype(np.float16) for i in range(8)]
        for b in range(NBLK):
            cs = slice(b * BLK, (b + 1) * BLK)
            pack[b, :, 0, :] = xT[0:128, cs]
            pack[b, :, 1, :] = xT[128:256, cs]
            pack[b, :, 2, :] = mT[1][:, cs]
            pack[b, :, 3, :], pack[b, :, 4, :] = mT[2][0:128, cs], mT[2][128:256, cs]
            for k in range(4):
                pack[b, :, 5 + k, :] = mT[3][k * 128:(k + 1) * 128, cs]
            pack[b, :, 9, :], pack[b, :, 10, :] = mT[4][0:128, cs], mT[4][128:256, cs]
            pack[b, :, 11, :] = mT[5][:, cs]
        # slot-staggered m678 planes: slot i masks {m6(i), m7(i-1), m8(i-2)};
        # plane 5 is block 3's diagonal {m6(3), m7(3), m8(3)} for the tail ladder
        for i in range(NSLOT - 1):
            if i < NBLK:
                m678[i, 0:64, 0, :] = mT[6][:, i * BLK:(i + 1) * BLK]
            if 0 <= i - 1 < NBLK:
                m678[i, 64:96, 0, :] = mT[7][:, (i - 1) * BLK:i * BLK]
            if 0 <= i - 2 < NBLK:
                m678[i, 96:112, 0, :] = mT[8][:, (i - 2) * BLK:(i - 1) * BLK]
        lb = slice((NBLK - 1) * BLK, NBLK * BLK)
        m678[5, 0:64, 0, :] = mT[6][:, lb]
        m678[5, 64:96, 0, :] = mT[7][:, lb]
        m678[5, 96:112, 0, :] = mT[8][:, lb]
        in_maps.append({"pack": pack, "M678": m678, **shared})
    return in_maps


def kernel(**inputs) -> np.ndarray:
    from concourse.bass_utils import run_bass_kernel_spmd

    nc = _get_program()
    in_maps = _host_prep(inputs)
    res = run_bass_kernel_spmd(nc, in_maps, list(range(NCORES)))
    out = np.empty((BATCH, DIMS[-1]), dtype=np.float32)
    for c in range(NCORES):
        out[c * SHARD:(c + 1) * SHARD, :] = res.results[c]["outT"].T
    return out
